# revision 1
# baseline (speedup 1.0000x reference)
"""GQA attention kernel for 8 TRN2 NeuronCores (tensor-parallel over heads).

Problem: B=2, S=2048, D=2048, HQ=32, HKV=8, HD=64, ALiBi + additive mask,
softmax, out-projection.  Each core owns 4 q-heads (= 1 kv head); each core
computes a full-shape partial of the output (its heads' contribution through
wo), and the host sums the 8 partials.

Layout strategy (per core):
  - all matmuls in float32r (TF32-like, 1 cycle/row at N>=256)
  - logits computed TRANSPOSED: logitsT[n, m] = kaug.T @ qaug with the
    contraction dim augmented by 2 rows that add alibi slope*(n-m) and a
    per-query stabilizer -c[m] for free:
       kaug = [kT(64); n; 1]            (shared by all 4 heads)
       qaug_h = [qT_h(64); slope_h; -slope_h*m - c_h[m]]
  - PT = exp(logitsT) ; AV matmul uses vaug = [v | ones] so the ones column
    accumulates the softmax denominators in psum row 64.
  - normalization folded into the OT eviction (DVE multiply by broadcast
    reciprocal), odd heads DMA-shifted to partitions 64:127 so the
    o-projection reads one contiguous [128, m] stationary per head-pair.
  - causal masks: dead logit tiles are skipped entirely; diagonal-crossing
    tiles get one of ceil(MC/128) precomputed [128, MC] additive patterns.
"""

import os
import sys

sys.path.insert(0, "/opt/trn_rl_repo")

import numpy as np

NEG = -1e9


# ---------------------------------------------------------------------------
# device program builder
# ---------------------------------------------------------------------------

def build_program(cfg):
    import concourse.bass as bass  # noqa: F401
    import concourse.mybir as mybir
    import concourse.tile as tile
    from concourse import bacc

    f32 = mybir.dt.float32
    f32r = mybir.dt.float32r

    B, S, D = cfg["B"], cfg["S"], cfg["D"]
    HLOC, HD = cfg["HLOC"], cfg["HD"]
    MC = cfg["MC"]                    # m-chunk (<= 512, psum bank)
    MPAIR = 2 * MC                    # exp / AV / normalize granularity
    causal = cfg["causal"]
    generic_mask = cfg["generic_mask"]

    DQ = HLOC * HD                    # local q dims (256)
    NKT = D // 128                    # contraction k-tiles for projections
    NNT = S // 128                    # n-tiles (keys)
    NMC = S // MC                     # m-chunks per b
    NPAIR = S // MPAIR                # m-pairs per b
    NHP = HLOC // 2                   # head pairs
    NPAT = MC // 128                  # diagonal mask patterns
    NEC = D // MC                     # out-proj e-chunks
    NMT = S // 128                    # out-proj m-tiles

    nc = bacc.Bacc("TRN2", target_bir_lowering=False, debug=False)

    xT_d = nc.dram_tensor("xT", [D, B, S], f32, kind="ExternalInput")
    wq_d = nc.dram_tensor("wqT", [D, DQ], f32, kind="ExternalInput")
    wkv_d = nc.dram_tensor("wkvT", [D, 2 * HD], f32, kind="ExternalInput")
    wo_d = nc.dram_tensor("woT", [DQ, D], f32, kind="ExternalInput")
    kaug_d = nc.dram_tensor("kaug_ext", [2, S], f32, kind="ExternalInput")
    qaug_d = nc.dram_tensor("qaug_ext", [HLOC, 2, S], f32, kind="ExternalInput")
    ident_d = nc.dram_tensor("ident", [64, 64], f32, kind="ExternalInput")
    if causal:
        mpat_d = nc.dram_tensor("maskpat", [128, 128], f32, kind="ExternalInput")
    if generic_mask:
        maskT_d = nc.dram_tensor("maskT", [S, S], f32, kind="ExternalInput")
    out_d = nc.dram_tensor("out", [B, S, D], f32, kind="ExternalOutput")
    debug = cfg.get("debug", False)
    if debug:
        dbg = {}
        for nm, shape in [("dbg_qaug0", [66, S]), ("dbg_kaug", [66, S]),
                          ("dbg_vaug", [128, (S // 128) * (HD + 1)]),
                          ("dbg_otu", [65, 2 * MC]), ("dbg_rbc", [128, 2 * MC]),
                          ("dbg_OT0", [128, (HLOC // 2) * 2 * MC])]:
            dbg[nm] = nc.dram_tensor(nm, shape, f32, kind="ExternalOutput")

    def live(nt, mc):
        """is logitsT tile (keys nt*128.., queries mc*MC..) not fully masked"""
        if not causal:
            return True
        return nt * 128 <= mc * MC + MC - 1

    def crossing(nt, mc):
        """does the tile cross the causal diagonal (needs mask pattern)"""
        if not causal:
            return False
        return live(nt, mc) and nt * 128 + 127 > mc * MC

    with tile.TileContext(nc) as tc:
        with tc.tile_pool(name="res", bufs=1) as res, \
             tc.tile_pool(name="sbp", bufs=3) as sbp, \
             tc.tile_pool(name="ps", bufs=2, space="PSUM") as ps:

            # ---- resident tiles ------------------------------------------
            wq_sb = res.tile([128, NKT, DQ], f32r, tag="wq")
            wkv_sb = res.tile([128, NKT, 2 * HD], f32r, tag="wkv")
            # interleave quarter-loads of wq/wkv so the first k-tiles land fast
            qtr = NKT // 4
            for qi in range(4):
                sl = slice(qi * qtr * 128, (qi + 1) * qtr * 128)
                nc.sync.dma_start(
                    wq_sb[:, qi * qtr:(qi + 1) * qtr, :],
                    wq_d.ap()[sl, :]
                    .rearrange("(kt p) q -> p kt q", p=128).bitcast(f32r))
                nc.sync.dma_start(
                    wkv_sb[:, qi * qtr:(qi + 1) * qtr, :],
                    wkv_d.ap()[sl, :]
                    .rearrange("(kt p) q -> p kt q", p=128).bitcast(f32r))
            wo_sb = res.tile([128, NHP, D], f32r, tag="wo")
            ident_sb = res.tile([64, 64], f32, tag="ident")
            nc.sync.dma_start(ident_sb[:], ident_d.ap()[:])
            if causal:
                mpat_sb = res.tile([128, 128], f32, tag="mpat")
                nc.sync.dma_start(mpat_sb[:], mpat_d.ap()[:])

            kaug = res.tile([66, S], f32r, tag="kaug")
            nc.sync.dma_start(kaug[64:66, :], kaug_d.ap()[:].bitcast(f32r))
            qaug = [res.tile([66, S], f32r, tag=f"qaug{h}", name=f"qaug{h}")
                    for h in range(HLOC)]
            for h in range(HLOC):
                nc.sync.dma_start(qaug[h][64:66, :], qaug_d.ap()[h].bitcast(f32r))
            vaug = res.tile([128, NNT, HD + 1], f32r, tag="vaug")
            nc.vector.memset(vaug[:].bitcast(f32), 1.0)
            vt_sb = res.tile([64, S], f32, tag="vt")
            OT_sb = [res.tile([128, NHP, MPAIR], f32r, tag=f"OT{p}", name=f"OT{p}")
                     for p in range(NPAIR)]

            for _rep in range(cfg.get("reps", 1)):
              for b in range(B):
                # ---- projections: qT, kT, vT for this b ------------------
                for mc in range(NMC):
                    mco = mc * MC
                    qp = ps.tile([128, NHP * MC], f32, tag="qk")
                    kvp = ps.tile([128, MC], f32, tag="av")
                    KQ = 4  # k-tiles per xt DMA
                    for ktq in range(NKT // KQ):
                        xt = sbp.tile([128, KQ, MC], f32r, tag="xt", bufs=4)
                        nc.sync.dma_start(
                            xt[:], xT_d.ap()[ktq * KQ * 128:(ktq + 1) * KQ * 128,
                                             b, mco:mco + MC]
                            .rearrange("(k p) m -> p k m", p=128).bitcast(f32r))
                        for kq in range(KQ):
                            kt = ktq * KQ + kq
                            st, sp = (kt == 0), (kt == NKT - 1)
                            for hp in range(NHP):
                                nc.tensor.matmul(
                                    qp[:, hp * MC:(hp + 1) * MC],
                                    wq_sb[:, kt, hp * 128:(hp + 1) * 128],
                                    xt[:, kq], start=st, stop=sp)
                            nc.tensor.matmul(kvp[:], wkv_sb[:, kt, :], xt[:, kq],
                                             start=st, stop=sp)
                    # evictions
                    for hp in range(NHP):
                        # even head of the pair: psum rows 0:64 -> qaug rows 0:64
                        nc.vector.tensor_copy(qaug[2 * hp][0:64, mco:mco + MC],
                                              qp[0:64, hp * MC:(hp + 1) * MC])
                        # odd head: rows 64:128, engine-copy then DMA shift
                        qtmp = sbp.tile([128, MC], f32r, tag="tmp", bufs=2)
                        nc.vector.tensor_copy(qtmp[64:128, :],
                                               qp[64:128, hp * MC:(hp + 1) * MC])
                        nc.sync.dma_start(qaug[2 * hp + 1][0:64, mco:mco + MC],
                                          qtmp[64:128, :])
                    nc.vector.tensor_copy(kaug[0:64, mco:mco + MC], kvp[0:64, :])
                    vtmp = sbp.tile([128, MC], f32, tag="tmp", bufs=2)
                    nc.vector.tensor_copy(vtmp[64:128, :], kvp[64:128, :])
                    nc.sync.dma_start(vt_sb[0:64, mco:mco + MC], vtmp[64:128, :])

                # ---- transpose vT -> v (vaug) ----------------------------
                # groups of 8 n-tiles per psum tile
                for g in range((NNT + 7) // 8):
                    nts = range(g * 8, min((g + 1) * 8, NNT))
                    vtp = ps.tile([128, 512], f32, tag="av")
                    for j, nt in enumerate(nts):
                        nc.tensor.transpose(
                            vtp[:, j * 64:(j + 1) * 64],
                            vt_sb[0:64, nt * 128:(nt + 1) * 128], ident_sb[:])
                    nc.vector.tensor_copy(vaug[:, nts.start:nts.stop, 0:HD],
                                            vtp[:, 0:64 * len(nts)].rearrange(
                                                "p (t d) -> p t d", d=64))

                if debug and b == 0:
                    nc.sync.dma_start(dbg["dbg_qaug0"].ap()[:],
                                      qaug[0][:].bitcast(f32))
                    nc.sync.dma_start(dbg["dbg_kaug"].ap()[:],
                                      kaug[:].bitcast(f32))
                    nc.sync.dma_start(
                        dbg["dbg_vaug"].ap()[:],
                        vaug[:].rearrange("p a b -> p (a b)").bitcast(f32))

                # ---- attention (pair-outer) + interleaved out-proj -------
                for pair in range(NPAIR):
                    po = pair * MPAIR
                    for h in range(HLOC):
                        hp, odd = h // 2, h % 2
                        av = [ps.tile([128, MC], f32, tag="av", name=f"av{c}")
                              for c in range(2)]
                        nlive = [nt for nt in range(NNT)
                                 if live(nt, 2 * pair) or live(nt, 2 * pair + 1)]
                        for nt in nlive:
                            qk = ps.tile([128, MPAIR], f32, tag="qk")
                            pt_t = sbp.tile([128, MPAIR], f32r, tag="pt", bufs=5)
                            ch_live = [c for c in range(2) if live(nt, 2 * pair + c)]
                            offs = {}
                            for c in ch_live:
                                mc = 2 * pair + c
                                # cols [0, o) of this chunk are fully masked
                                o = max(0, nt * 128 - mc * MC) if causal else 0
                                offs[c] = o
                                lo = c * MC + o
                                nc.tensor.matmul(
                                    qk[:, lo:(c + 1) * MC],
                                    kaug[:, nt * 128:(nt + 1) * 128],
                                    qaug[h][:, mc * MC + o:(mc + 1) * MC],
                                    start=True, stop=True)
                                if generic_mask:
                                    mtile = sbp.tile([128, MC], f32, tag="mt")
                                    nc.sync.dma_start(
                                        mtile[:],
                                        maskT_d.ap()[nt * 128:(nt + 1) * 128,
                                                     mc * MC:(mc + 1) * MC])
                                    nc.vector.tensor_add(
                                        qk[:, c * MC:(c + 1) * MC],
                                        qk[:, c * MC:(c + 1) * MC], mtile[:])
                                elif crossing(nt, mc):
                                    # triangular band on cols [o, o+128)
                                    nc.vector.tensor_add(
                                        qk[:, lo:lo + 128],
                                        qk[:, lo:lo + 128], mpat_sb[:])
                            c0, c1 = ch_live[0], ch_live[-1] + 1
                            o0 = offs[c0]
                            if o0:
                                nc.vector.memset(pt_t[:, c0 * MC:c0 * MC + o0].bitcast(f32), 0.0)
                            nc.scalar.activation(
                                pt_t[:, c0 * MC + o0:c1 * MC],
                                qk[:, c0 * MC + o0:c1 * MC],
                                mybir.ActivationFunctionType.Exp)
                            for c in ch_live:
                                mc = 2 * pair + c
                                last_nt = (mc * MC + MC - 1) // 128 if causal else NNT - 1
                                nc.tensor.matmul(
                                    av[c][0:HD + 1, :],
                                    vaug[:, nt, :], pt_t[:, c * MC:(c + 1) * MC],
                                    start=(nt == 0), stop=(nt == last_nt))
                        # evict unnormalized OT+sums immediately (frees psum)
                        otu = sbp.tile([65, MPAIR], f32, tag="otu", bufs=2)
                        for c in range(2):
                            nc.vector.tensor_copy(otu[0:65, c * MC:(c + 1) * MC],
                                                  av[c][0:HD + 1, :])
                        srow = sbp.tile([1, MPAIR], f32, tag="srow", bufs=2)
                        nc.sync.dma_start(srow[0:1, :], otu[64:65, :])
                        rbc = sbp.tile([128, MPAIR], f32, tag="rbc", bufs=2)
                        nc.gpsimd.partition_broadcast(rbc[:], srow[0:1, :])
                        nc.vector.reciprocal(rbc[:], rbc[:])
                        if debug and b == 0 and pair == 0 and h == 0:
                            nc.sync.dma_start(dbg["dbg_otu"].ap()[:], otu[:])
                            nc.sync.dma_start(dbg["dbg_rbc"].ap()[:], rbc[:])
                        if not odd:
                            nc.vector.tensor_mul(
                                OT_sb[pair][0:64, hp, :],
                                otu[0:64, :], rbc[0:64, :])
                        else:
                            nc.sync.dma_start(OT_sb[pair][64:128, hp, :],
                                              otu[0:64, :].bitcast(f32r))
                            nc.vector.tensor_mul(
                                OT_sb[pair][64:128, hp, :],
                                OT_sb[pair][64:128, hp, :], rbc[64:128, :])
                    if debug and b == 0 and pair == 0:
                        nc.sync.dma_start(
                            dbg["dbg_OT0"].ap()[:],
                            OT_sb[0][:].rearrange("p a b -> p (a b)").bitcast(f32))
                    # ---- out-projection for this pair's m-tiles ----------
                    if b == 0 and pair == 0:
                        nc.sync.dma_start(
                            wo_sb[:],
                            wo_d.ap()[:].rearrange("(hp p) e -> p hp e",
                                                   p=128).bitcast(f32r))
                    for mtl in range(MPAIR // 128):
                        mt = pair * (MPAIR // 128) + mtl
                        ob = sbp.tile([128, D], f32, tag="ob", bufs=2)
                        for ec in range(NEC):
                            op = ps.tile([128, MC], f32, tag="pp")
                            for hp in range(NHP):
                                nc.tensor.matmul(
                                    op[:],
                                    OT_sb[pair][:, hp, mtl * 128:(mtl + 1) * 128],
                                    wo_sb[:, hp, ec * MC:(ec + 1) * MC],
                                    start=(hp == 0), stop=(hp == NHP - 1))
                            nc.vector.tensor_copy(ob[:, ec * MC:(ec + 1) * MC],
                                                  op[:])
                        nc.sync.dma_start(
                            out_d.ap()[b, mt * 128:(mt + 1) * 128, :], ob[:])

    nc.compile()
    return nc


# ---------------------------------------------------------------------------
# host side
# ---------------------------------------------------------------------------

def _analyze_mask(mask2d, S):
    """classify mask; return (causal, zeros, n_lo, n_hi)"""
    masked = mask2d < -1e8
    if not masked.any():
        return False, True, np.zeros(S, np.int64), np.full(S, S - 1, np.int64)
    tri = np.triu(np.ones((S, S), bool), 1)
    if (masked == tri).all() and (mask2d[~masked] == 0).all():
        return True, False, np.zeros(S, np.int64), np.arange(S)
    allowed = ~masked
    # guard fully-masked rows (keep index 0; softmax row is garbage anyway)
    any_allowed = allowed.any(axis=1)
    idx = np.arange(S)[None, :]
    n_hi = np.where(any_allowed, np.where(allowed, idx, -1).max(axis=1), 0)
    n_lo = np.where(any_allowed, np.where(allowed, idx, S).min(axis=1), 0)
    return False, False, n_lo, n_hi


def _make_inputs_for_core(core, x, wq, wk, wv, wo, slopes, mask, cfg):
    B, S, D, HLOC, HD = cfg["B"], cfg["S"], cfg["D"], cfg["HLOC"], cfg["HD"]
    MC = cfg["MC"]
    h0 = core * HLOC
    kv = core  # one kv head per core
    scale = 1.0 / np.sqrt(HD)

    xT = np.ascontiguousarray(x.transpose(2, 0, 1))                 # [D,B,S]
    wqT = np.ascontiguousarray((wq[h0 * HD:(h0 + HLOC) * HD] * scale).T)
    wkvT = np.ascontiguousarray(
        np.concatenate([wk[kv * HD:(kv + 1) * HD], wv[kv * HD:(kv + 1) * HD]],
                       axis=0).T)                                   # [D,128]
    woT = np.ascontiguousarray(wo[:, h0 * HD:(h0 + HLOC) * HD].T)   # [DQ,D]

    n = np.arange(S, dtype=np.float32)
    kaug_ext = np.stack([n, np.ones(S, np.float32)])                # [2,S]

    qaug_ext = np.zeros((HLOC, 2, S), np.float32)
    for i in range(HLOC):
        sl = float(slopes[h0 + i])
        # stabilizer c[m] = max over allowed n of slope*(n-m), clipped >= 0
        c = np.maximum(0.0, np.maximum(sl * (cfg["n_hi"] - n),
                                       sl * (cfg["n_lo"] - n)))
        qaug_ext[i, 0, :] = sl
        qaug_ext[i, 1, :] = -sl * n - c

    ident = np.eye(64, dtype=np.float32)

    ins = {"xT": xT, "wqT": wqT, "wkvT": wkvT, "woT": woT,
           "kaug_ext": kaug_ext, "qaug_ext": qaug_ext, "ident": ident}
    if cfg["causal"]:
        ii = np.arange(128)[:, None]
        jj = np.arange(128)[None, :]
        ins["maskpat"] = np.where(ii > jj, NEG, 0.0).astype(np.float32)
    if cfg["generic_mask"]:
        ins["maskT"] = np.ascontiguousarray(mask[0, 0].T)
    return ins


def _host_reference_partial(core, inputs, cfg):
    """numpy emulation of one core's partial (for testing the builder)"""
    x, wq, wk, wv, wo = (inputs[k] for k in ("x", "wq", "wk", "wv", "wo"))
    slopes, mask = inputs["slopes"], inputs["mask"]
    B, S, HLOC, HD = cfg["B"], cfg["S"], cfg["HLOC"], cfg["HD"]
    h0, kvh = core * HLOC, core
    q = (x @ wq.T)[..., h0 * HD:(h0 + HLOC) * HD]
    k = (x @ wk.T)[..., kvh * HD:(kvh + 1) * HD]
    v = (x @ wv.T)[..., kvh * HD:(kvh + 1) * HD]
    out = np.zeros_like(x)
    rel = (np.arange(S)[None, :] - np.arange(S)[:, None]).astype(np.float32)
    for h in range(HLOC):
        qh = q[..., h * HD:(h + 1) * HD] / np.sqrt(HD)
        lg = np.einsum('bmd,bnd->bmn', qh, k)
        lg += slopes[h0 + h] * rel[None] + mask[0]
        lg -= lg.max(axis=-1, keepdims=True)
        p = np.exp(lg)
        p /= p.sum(axis=-1, keepdims=True)
        oh = np.einsum('bmn,bnd->bmd', p, v)
        out += oh @ wo[:, (h0 + h) * HD:(h0 + h + 1) * HD].T
    return out


def kernel(x, wq, wk, wv, wo, slopes, mask, _debug_sim=False):
    from concourse.bass_utils import run_bass_kernel_spmd

    x = np.asarray(x, dtype=np.float32)
    wq = np.asarray(wq, dtype=np.float32)
    wk = np.asarray(wk, dtype=np.float32)
    wv = np.asarray(wv, dtype=np.float32)
    wo = np.asarray(wo, dtype=np.float32)
    slopes = np.asarray(slopes, dtype=np.float32)
    mask = np.asarray(mask, dtype=np.float32)

    B, S, D = x.shape
    HQ = 32
    HD = D // HQ
    n_cores = 8
    HLOC = HQ // n_cores

    causal, zeros, n_lo, n_hi = _analyze_mask(mask[0, 0], S)
    cfg = dict(B=B, S=S, D=D, HLOC=HLOC, HD=HD, MC=512,
               causal=causal, generic_mask=not (causal or zeros),
               n_lo=n_lo, n_hi=n_hi)

    nc = build_program(cfg)
    in_maps = [_make_inputs_for_core(c, x, wq, wk, wv, wo, slopes, mask, cfg)
               for c in range(n_cores)]
    res = run_bass_kernel_spmd(nc, in_maps, core_ids=list(range(n_cores)))
    out = np.zeros((B, S, D), np.float32)
    for c in range(n_cores):
        out += res.results[c]["out"]
    return out


if __name__ == "__main__":
    # quick self-test with a tiny config through CoreSim
    pass



# revision 33
# speedup vs baseline: 1.5059x; 1.5059x over previous
"""GQA attention kernel for 8 TRN2 NeuronCores (tensor-parallel over heads).

Problem: B=2, S=2048, D=2048, HQ=32, HKV=8, HD=64, ALiBi + additive mask,
softmax, out-projection.  Each core owns 4 q-heads (= 1 kv head); each core
computes a full-shape partial of the output (its heads' contribution through
wo), and the host sums the 8 partials.

v2 layout strategy (per core):
  - data path in bf16 (x, wq/wk/wv, wo, v, exp(logits), attention outputs,
    DRAM output partial); psum stays f32.  ALiBi aug rows need f32 range
    (slope*m up to ~2e3), so the logits matmul runs f32r on f32 qaug/kaug
    whose data rows are written from the f32 projection psum.
  - logits computed TRANSPOSED: logitsT[n, m] = kaug.T @ qaug with the
    contraction dim augmented by 2 rows that add alibi slope*(n-m) and a
    per-query stabilizer -c[m] for free:
       kaug = [kT(64); n; 1]            (shared by all 4 heads)
       qaug_h = [qT_h(64); slope_h; -slope_h*m - c_h[m]]
  - PT = exp(logitsT) in bf16; AV matmul uses vaug = [v | ones] so the ones
    column accumulates softmax denominators in psum row 64.  AV matmuls are
    column-trimmed to the causal region with per-diagonal-block stop flags.
  - normalization: denominator row is copied out of psum, partition-broadcast
    (Pool), and divided into the AV psum during the bf16 eviction (DVE).
    Odd heads are DMA-shifted to partitions 64:127 so the o-projection reads
    one contiguous [128, m] stationary per head pair.
  - out-projection is split into per-128-query units and software-pipelined:
    units are interleaved into the NEXT attention/projection phase so the PE
    never waits on the normalize chain.
  - causal masks: dead logit tiles are skipped; diagonal-crossing tiles get a
    precomputed [128,128] additive pattern (DVE/Pool alternating).
"""

import sys

sys.path.insert(0, "/opt/trn_rl_repo")

import numpy as np

NEG = -1e9


# ---------------------------------------------------------------------------
# device program builder
# ---------------------------------------------------------------------------

def build_program(cfg):
    import concourse.bass as bass  # noqa: F401
    import concourse.mybir as mybir
    import concourse.tile as tile
    from concourse import bacc

    f32 = mybir.dt.float32
    f32r = mybir.dt.float32r
    bf16 = mybir.dt.bfloat16

    B, S, D = cfg["B"], cfg["S"], cfg["D"]
    HLOC, HD = cfg["HLOC"], cfg["HD"]
    MC = cfg["MC"]                    # m-chunk (<= 512, psum bank)
    MPAIR = 2 * MC                    # exp / AV / normalize granularity
    causal = cfg["causal"]
    generic_mask = cfg["generic_mask"]

    DQ = HLOC * HD                    # local q dims (256)
    DKV = 2 * HD                      # local kv dims (128)
    NKT = D // 128                    # contraction k-tiles for projections
    NNT = S // 128                    # n-tiles (keys)
    NMC = S // MC                     # m-chunks per b
    NPAIR = S // MPAIR                # m-pairs per b
    NHP = HLOC // 2                   # head pairs
    NEC = D // MC                     # out-proj e-chunks

    nc = bacc.Bacc("TRN2", target_bir_lowering=False, debug=False)

    xT_d = nc.dram_tensor("xT", [D, B, S], bf16, kind="ExternalInput")
    wqkv_d = nc.dram_tensor("wqkvT", [D, DQ + DKV], bf16, kind="ExternalInput")
    wo_d = nc.dram_tensor("woT", [DQ, D], bf16, kind="ExternalInput")
    kaug_d = nc.dram_tensor("kaug_ext", [2, S], f32, kind="ExternalInput")
    qaug_d = nc.dram_tensor("qaug_ext", [HLOC, 2, S], f32, kind="ExternalInput")
    ident_d = nc.dram_tensor("ident", [64, 64], bf16, kind="ExternalInput")
    if causal:
        # mask pattern applied on the PE: qk += ident128.T @ mpat
        ident128_d = nc.dram_tensor("ident128", [128, 128], bf16,
                                    kind="ExternalInput")
        mpat_d = nc.dram_tensor("maskpat", [128, 128], bf16,
                                kind="ExternalInput")
    if generic_mask:
        maskT_d = nc.dram_tensor("maskT", [S, S], f32, kind="ExternalInput")
    out_d = nc.dram_tensor("out", [B, S, D], bf16, kind="ExternalOutput")

    def live(nt, mc):
        """is logitsT tile (keys nt*128.., queries mc*MC..) not fully masked"""
        if not causal:
            return True
        return nt * 128 <= mc * MC + MC - 1

    def crossing(nt, mc):
        """does the tile cross the causal diagonal (needs mask pattern)"""
        if not causal:
            return False
        return live(nt, mc) and nt * 128 + 127 > mc * MC

    with tile.TileContext(nc) as tc:
        with tc.tile_pool(name="res", bufs=1) as res, \
             tc.tile_pool(name="dbl", bufs=2) as dbl, \
             tc.tile_pool(name="sbp", bufs=3) as sbp, \
             tc.tile_pool(name="ps", bufs=1, space="PSUM") as ps:

            # ---- resident weights ----------------------------------------
            # wqkv quarters go on the SP queue (needed by the first matmul);
            # everything else loads via the ACT queue so the first xt DMA
            # isn't stuck behind resident loads on the in-order SP queue.
            wqkv_sb = res.tile([128, NKT, DQ + DKV], bf16, tag="wqkv")
            qtr = NKT // 4

            def _wqkv_quarter(qi):
                sl = slice(qi * qtr * 128, (qi + 1) * qtr * 128)
                nc.sync.dma_start(
                    wqkv_sb[:, qi * qtr:(qi + 1) * qtr, :],
                    wqkv_d.ap()[sl, :]
                    .rearrange("(kt p) q -> p kt q", p=128))

            # quarter 0 now; 1-3 deferred until after the first xt DMA so the
            # first projection matmul isn't stuck behind them on DMA_ENGINES
            _wqkv_quarter(0)
            deferred = [lambda qi=qi: _wqkv_quarter(qi) for qi in range(1, 4)]
            wo_sb = res.tile([128, NHP, D], bf16, tag="wo")
            nc.scalar.dma_start(
                wo_sb[:],
                wo_d.ap()[:].rearrange("(hp p) e -> p hp e", p=128))
            ident_sb = res.tile([64, 64], bf16, tag="ident")
            nc.scalar.dma_start(ident_sb[:], ident_d.ap()[:])
            if causal:
                ident128_sb = res.tile([128, 128], bf16, tag="ident128")
                nc.scalar.dma_start(ident128_sb[:], ident128_d.ap()[:])
                mpat_sb = res.tile([128, 128], bf16, tag="mpat")
                nc.scalar.dma_start(mpat_sb[:], mpat_d.ap()[:])

            # per-b double-buffered activations (allocated inside the b loop)
            state = {}
            alt = {"i": 0}  # DVE/Pool alternation for mask adds + oproj evicts

            def proj_mc(b, mc):
                """projections for m-chunk mc of batch b"""
                kaug, qaug, vt = state["kaug"], state["qaug"], state["vt"]
                mco = mc * MC
                qp = [ps.tile([128, MC], f32, tag="qk", bufs=4,
                              name=f"qp{hp}") for hp in range(NHP)]
                kvp = ps.tile([128, MC], f32, tag="pp", bufs=2, name="kvp")
                KQ = 4  # k-tiles per xt DMA
                for ktq in range(NKT // KQ):
                    xt = sbp.tile([128, KQ, MC], bf16, tag="xt", bufs=4)
                    nc.sync.dma_start(
                        xt[:], xT_d.ap()[ktq * KQ * 128:(ktq + 1) * KQ * 128,
                                         b, mco:mco + MC]
                        .rearrange("(k p) m -> p k m", p=128))
                    while deferred:
                        deferred.pop(0)()
                    for kq in range(KQ):
                        kt = ktq * KQ + kq
                        st, sp = (kt == 0), (kt == NKT - 1)
                        for hp in range(NHP):
                            nc.tensor.matmul(
                                qp[hp][:],
                                wqkv_sb[:, kt, hp * 128:(hp + 1) * 128],
                                xt[:, kq], start=st, stop=sp)
                        nc.tensor.matmul(kvp[:], wqkv_sb[:, kt, DQ:DQ + DKV],
                                         xt[:, kq], start=st, stop=sp)
                # evictions, spread across DVE/ACT so qp frees fast
                # (GPSIMD cannot access PSUM)
                for hp in range(NHP):
                    # even head of the pair: psum rows 0:64 -> qaug rows 0:64
                    nc.vector.tensor_copy(qaug[2 * hp][0:64, mco:mco + MC],
                                          qp[hp][0:64, :])
                    # odd head: rows 64:128, engine-copy then DMA shift
                    qtmp = sbp.tile([128, MC], f32r, tag="qtmp", bufs=4,
                                    name="qtmp")
                    nc.vector.tensor_copy(qtmp[64:128, :], qp[hp][64:128, :])
                    nc.sync.dma_start(qaug[2 * hp + 1][0:64, mco:mco + MC],
                                      qtmp[64:128, :])
                nc.vector.tensor_copy(kaug[0:64, mco:mco + MC], kvp[0:64, :])
                vtmp = sbp.tile([128, MC], bf16, tag="vtmp", bufs=2,
                                name="vtmp")
                nc.scalar.activation(vtmp[64:128, :], kvp[64:128, :],
                                     mybir.ActivationFunctionType.Copy)
                nc.sync.dma_start(vt[0:64, mco:mco + MC], vtmp[64:128, :])

            def vtrans(b):
                """transpose vT -> v (vaug), groups of 8 n-tiles per psum"""
                vt, vaug = state["vt"], state["vaug"]
                for g in range((NNT + 7) // 8):
                    nts = range(g * 8, min((g + 1) * 8, NNT))
                    vtp = ps.tile([128, 512], bf16, tag="pp", bufs=2,
                                  name="vtp")
                    for j, nt in enumerate(nts):
                        nc.tensor.transpose(
                            vtp[:, j * 64:(j + 1) * 64],
                            vt[0:64, nt * 128:(nt + 1) * 128], ident_sb[:])
                    nc.vector.tensor_copy(vaug[:, nts.start:nts.stop, 0:HD],
                                          vtp[:, 0:64 * len(nts)].rearrange(
                                              "p (t d) -> p t d", d=64))

            def attn_head(b, pair, h):
                kaug, qaug, vaug = state["kaug"], state["qaug"], state["vaug"]
                OT = state["OT"]
                hp, odd = h // 2, h % 2
                av = [ps.tile([128, MC], f32, tag="av", bufs=2,
                              name=f"av{c}") for c in range(2)]
                nlive = [nt for nt in range(NNT)
                         if live(nt, 2 * pair) or live(nt, 2 * pair + 1)]
                last_nt = nlive[-1]

                def emit_av(nt, c, pt):
                    st = (nt == 0)
                    if causal:
                        mc = 2 * pair + c
                        # columns whose diagonal (last) tile is nt
                        sl = max(0, nt * 128 - mc * MC)
                        sh = min(MC, nt * 128 + 128 - mc * MC)
                        if sh > sl:
                            nc.tensor.matmul(
                                av[c][0:HD + 1, sl:sh],
                                vaug[:, nt, :], pt[:, sl:sh],
                                start=st, stop=True,
                                skip_group_check=True)
                            if sh < MC:
                                nc.tensor.matmul(
                                    av[c][0:HD + 1, sh:MC],
                                    vaug[:, nt, :], pt[:, sh:MC],
                                    start=st, stop=False,
                                    skip_group_check=True)
                        else:
                            nc.tensor.matmul(
                                av[c][0:HD + 1, :], vaug[:, nt, :], pt[:],
                                start=st, stop=False,
                                skip_group_check=True)
                    else:
                        nc.tensor.matmul(
                            av[c][0:HD + 1, :], vaug[:, nt, :], pt[:],
                            start=st, stop=(nt == last_nt))

                # software pipeline: AV runs LAG chunk-steps behind
                # logits/exp so the exp round-trip latency never blocks the
                # in-order PE queue
                LAG = 4
                fifo = []
                for nt in nlive:
                    for c in range(2):
                        mc = 2 * pair + c
                        if not live(nt, mc):
                            continue
                        o = max(0, nt * 128 - mc * MC) if causal else 0
                        qk = ps.tile([128, MC], f32, tag="qk", bufs=4,
                                     name="qk")
                        pt = sbp.tile([128, MC], bf16, tag="pt", bufs=6,
                                      name="pt")
                        cross = crossing(nt, mc)
                        nc.tensor.matmul(
                            qk[:, o:MC],
                            kaug[:, nt * 128:(nt + 1) * 128],
                            qaug[h][:, mc * MC + o:(mc + 1) * MC],
                            start=True, stop=not cross,
                            skip_group_check=cross)
                        if generic_mask:
                            mtile = sbp.tile([128, MC], f32, tag="mt",
                                             name="mt")
                            nc.sync.dma_start(
                                mtile[:],
                                maskT_d.ap()[nt * 128:(nt + 1) * 128,
                                             mc * MC:(mc + 1) * MC])
                            nc.vector.tensor_add(qk[:], qk[:], mtile[:])
                        elif cross:
                            # accumulate the causal pattern on the PE
                            nc.tensor.matmul(
                                qk[:, o:o + 128], ident128_sb[:],
                                mpat_sb[:], start=False, stop=True,
                                skip_group_check=True)
                        nc.scalar.activation(
                            pt[:, o:MC], qk[:, o:MC],
                            mybir.ActivationFunctionType.Exp)
                        fifo.append((nt, c, pt))
                        if len(fifo) > LAG:
                            emit_av(*fifo.pop(0))
                for item in fifo:
                    emit_av(*item)
                # normalize per chunk (chunk 0's chain overlaps chunk 1's
                # remaining AV matmuls): denom row -> broadcast -> divide
                # folded into the bf16 psum eviction
                qdiv = None
                if odd:
                    qdiv = sbp.tile([64, MPAIR], bf16, tag="qdiv", bufs=2,
                                    name="qdiv")
                for c in range(2):
                    cs = slice(c * MC, (c + 1) * MC)
                    srow = sbp.tile([1, MC], f32, tag="srow", bufs=4,
                                    name="srow")
                    nc.vector.tensor_copy(srow[0:1, :], av[c][HD:HD + 1, :])
                    nc.vector.reciprocal(srow[0:1, :], srow[0:1, :])
                    rbc = sbp.tile([128, MC], f32, tag="rbc", bufs=4,
                                   name="rbc")
                    nc.gpsimd.partition_broadcast(rbc[:], srow[0:1, :])
                    dst = qdiv[0:64, cs] if odd else OT[0:64, hp, cs]
                    nc.vector.tensor_mul(dst, av[c][0:64, :], rbc[0:64, :])
                if odd:
                    nc.sync.dma_start(OT[64:128, hp, :], qdiv[0:64, :])

            def oproj_mt(b, pair, OT, mtl):
                """out-projection for one 128-query tile of a pair"""
                mt = pair * (MPAIR // 128) + mtl
                ob = sbp.tile([128, D], bf16, tag="ob", bufs=2, name="ob")
                for ec in range(NEC):
                    op = ps.tile([128, MC], f32, tag="pp", bufs=2, name="op")
                    for hp in range(NHP):
                        nc.tensor.matmul(
                            op[:],
                            OT[:, hp, mtl * 128:(mtl + 1) * 128],
                            wo_sb[:, hp, ec * MC:(ec + 1) * MC],
                            start=(hp == 0), stop=(hp == NHP - 1))
                    if ec == 3:
                        nc.scalar.activation(
                            ob[:, ec * MC:(ec + 1) * MC], op[:],
                            mybir.ActivationFunctionType.Copy)
                    else:
                        nc.vector.tensor_copy(ob[:, ec * MC:(ec + 1) * MC],
                                              op[:])
                nc.sync.dma_start(
                    out_d.ap()[b, mt * 128:(mt + 1) * 128, :], ob[:])

            # ---- schedule: proj/attention with interleaved out-proj ------
            pending = []

            def drain(k):
                for _ in range(min(k, len(pending))):
                    pending.pop(0)()

            for _rep in range(cfg.get("reps", 1)):
                for b in range(B):
                    state["kaug"] = dbl.tile([66, S], f32r, tag="kaug",
                                             bufs=2, name="kaug")
                    nc.scalar.dma_start(state["kaug"][64:66, :],
                                        kaug_d.ap()[:].bitcast(f32r))
                    qaug = []
                    for h in range(HLOC):
                        t = dbl.tile([66, S], f32r, tag=f"qaug{h}", bufs=2,
                                     name=f"qaug{h}")
                        nc.scalar.dma_start(t[64:66, :],
                                            qaug_d.ap()[h].bitcast(f32r))
                        qaug.append(t)
                    state["qaug"] = qaug
                    state["vt"] = dbl.tile([64, S], bf16, tag="vt", bufs=2,
                                           name="vt")
                    state["vaug"] = dbl.tile([128, NNT, HD + 1], bf16,
                                             tag="vaug", bufs=2, name="vaug")
                    nc.vector.memset(state["vaug"][:], 1.0)

                    for mc in range(NMC):
                        proj_mc(b, mc)
                        drain(2)
                    vtrans(b)
                    drain(2)
                    for pair in range(NPAIR):
                        OT = dbl.tile([128, NHP, MPAIR], bf16, tag="OT",
                                      bufs=3, name="OT")
                        state["OT"] = OT
                        for h in range(HLOC):
                            attn_head(b, pair, h)
                            drain(2)
                        for mtl in range(MPAIR // 128):
                            pending.append(
                                lambda b=b, pair=pair, OT=OT, mtl=mtl:
                                oproj_mt(b, pair, OT, mtl))
                drain(len(pending))

    nc.compile()
    return nc


# ---------------------------------------------------------------------------
# host side
# ---------------------------------------------------------------------------

def _analyze_mask(mask2d, S):
    """classify mask; return (causal, zeros, n_lo, n_hi)"""
    masked = mask2d < -1e8
    if not masked.any():
        return False, True, np.zeros(S, np.int64), np.full(S, S - 1, np.int64)
    tri = np.triu(np.ones((S, S), bool), 1)
    if (masked == tri).all() and (mask2d[~masked] == 0).all():
        return True, False, np.zeros(S, np.int64), np.arange(S)
    allowed = ~masked
    # guard fully-masked rows (keep index 0; softmax row is garbage anyway)
    any_allowed = allowed.any(axis=1)
    idx = np.arange(S)[None, :]
    n_hi = np.where(any_allowed, np.where(allowed, idx, -1).max(axis=1), 0)
    n_lo = np.where(any_allowed, np.where(allowed, idx, S).min(axis=1), 0)
    return False, False, n_lo, n_hi


def _bf16(a):
    import ml_dtypes
    return np.ascontiguousarray(a).astype(ml_dtypes.bfloat16)


def _make_inputs_for_core(core, x, wq, wk, wv, wo, slopes, mask, cfg):
    B, S, D, HLOC, HD = cfg["B"], cfg["S"], cfg["D"], cfg["HLOC"], cfg["HD"]
    h0 = core * HLOC
    kv = core  # one kv head per core
    scale = 1.0 / np.sqrt(HD)

    xT = np.ascontiguousarray(x.transpose(2, 0, 1))                 # [D,B,S]
    wqkvT = np.concatenate(
        [wq[h0 * HD:(h0 + HLOC) * HD] * scale,
         wk[kv * HD:(kv + 1) * HD],
         wv[kv * HD:(kv + 1) * HD]], axis=0).T                      # [D,384]
    woT = np.ascontiguousarray(wo[:, h0 * HD:(h0 + HLOC) * HD].T)   # [DQ,D]

    n = np.arange(S, dtype=np.float32)
    kaug_ext = np.stack([n, np.ones(S, np.float32)])                # [2,S]

    qaug_ext = np.zeros((HLOC, 2, S), np.float32)
    for i in range(HLOC):
        sl = float(slopes[h0 + i])
        # stabilizer c[m] = max over allowed n of slope*(n-m), clipped >= 0
        c = np.maximum(0.0, np.maximum(sl * (cfg["n_hi"] - n),
                                       sl * (cfg["n_lo"] - n)))
        qaug_ext[i, 0, :] = sl
        qaug_ext[i, 1, :] = -sl * n - c

    ident = np.eye(64, dtype=np.float32)

    ins = {"xT": _bf16(xT), "wqkvT": _bf16(wqkvT), "woT": _bf16(woT),
           "kaug_ext": kaug_ext, "qaug_ext": qaug_ext,
           "ident": _bf16(ident)}
    if cfg["causal"]:
        ii = np.arange(128)[:, None]
        jj = np.arange(128)[None, :]
        ins["maskpat"] = _bf16(np.where(ii > jj, NEG, 0.0))
        ins["ident128"] = _bf16(np.eye(128))
    if cfg["generic_mask"]:
        ins["maskT"] = np.ascontiguousarray(mask[0, 0].T)
    return ins


def kernel(x, wq, wk, wv, wo, slopes, mask):
    from concourse.bass_utils import run_bass_kernel_spmd

    x = np.asarray(x, dtype=np.float32)
    wq = np.asarray(wq, dtype=np.float32)
    wk = np.asarray(wk, dtype=np.float32)
    wv = np.asarray(wv, dtype=np.float32)
    wo = np.asarray(wo, dtype=np.float32)
    slopes = np.asarray(slopes, dtype=np.float32)
    mask = np.asarray(mask, dtype=np.float32)

    B, S, D = x.shape
    HQ = 32
    HD = D // HQ
    n_cores = 8
    HLOC = HQ // n_cores

    causal, zeros, n_lo, n_hi = _analyze_mask(mask[0, 0], S)
    cfg = dict(B=B, S=S, D=D, HLOC=HLOC, HD=HD, MC=512,
               causal=causal, generic_mask=not (causal or zeros),
               n_lo=n_lo, n_hi=n_hi)

    nc = build_program(cfg)
    in_maps = [_make_inputs_for_core(c, x, wq, wk, wv, wo, slopes, mask, cfg)
               for c in range(n_cores)]
    res = run_bass_kernel_spmd(nc, in_maps, core_ids=list(range(n_cores)))
    out = np.zeros((B, S, D), np.float32)
    for c in range(n_cores):
        out += res.results[c]["out"].astype(np.float32)
    return out


if __name__ == "__main__":
    pass


# revision 38
# speedup vs baseline: 1.6051x; 1.0659x over previous
"""GQA attention kernel for 8 TRN2 NeuronCores (tensor-parallel over heads).

Problem: B=2, S=2048, D=2048, HQ=32, HKV=8, HD=64, ALiBi + additive mask,
softmax, out-projection.  Each core owns 4 q-heads (= 1 kv head); each core
computes a full-shape partial of the output (its heads' contribution through
wo), and the host sums the 8 partials.

v2 layout strategy (per core):
  - data path in bf16 (x, wq/wk/wv, wo, v, exp(logits), attention outputs,
    DRAM output partial); psum stays f32.  ALiBi aug rows need f32 range
    (slope*m up to ~2e3), so the logits matmul runs f32r on f32 qaug/kaug
    whose data rows are written from the f32 projection psum.
  - logits computed TRANSPOSED: logitsT[n, m] = kaug.T @ qaug with the
    contraction dim augmented by 2 rows that add alibi slope*(n-m) and a
    per-query stabilizer -c[m] for free:
       kaug = [kT(64); n; 1]            (shared by all 4 heads)
       qaug_h = [qT_h(64); slope_h; -slope_h*m - c_h[m]]
  - PT = exp(logitsT) in bf16; AV matmul uses vaug = [v | ones] so the ones
    column accumulates softmax denominators in psum row 64.  AV matmuls are
    column-trimmed to the causal region with per-diagonal-block stop flags.
  - normalization: denominator row is copied out of psum, partition-broadcast
    (Pool), and divided into the AV psum during the bf16 eviction (DVE).
    Odd heads are DMA-shifted to partitions 64:127 so the o-projection reads
    one contiguous [128, m] stationary per head pair.
  - out-projection is split into per-128-query units and software-pipelined:
    units are interleaved into the NEXT attention/projection phase so the PE
    never waits on the normalize chain.
  - causal masks: dead logit tiles are skipped; diagonal-crossing tiles get a
    precomputed [128,128] additive pattern (DVE/Pool alternating).
"""

import sys

sys.path.insert(0, "/opt/trn_rl_repo")

import numpy as np

NEG = -1e9


# ---------------------------------------------------------------------------
# device program builder
# ---------------------------------------------------------------------------

def build_program(cfg):
    import concourse.bass as bass  # noqa: F401
    import concourse.mybir as mybir
    import concourse.tile as tile
    from concourse import bacc

    f32 = mybir.dt.float32
    f32r = mybir.dt.float32r
    bf16 = mybir.dt.bfloat16

    B, S, D = cfg["B"], cfg["S"], cfg["D"]
    HLOC, HD = cfg["HLOC"], cfg["HD"]
    MC = cfg["MC"]                    # m-chunk (<= 512, psum bank)
    MPAIR = 2 * MC                    # exp / AV / normalize granularity
    causal = cfg["causal"]
    generic_mask = cfg["generic_mask"]

    DQ = HLOC * HD                    # local q dims (256)
    DKV = 2 * HD                      # local kv dims (128)
    NKT = D // 128                    # contraction k-tiles for projections
    NNT = S // 128                    # n-tiles (keys)
    NMC = S // MC                     # m-chunks per b
    NPAIR = S // MPAIR                # m-pairs per b
    NHP = HLOC // 2                   # head pairs
    NEC = D // MC                     # out-proj e-chunks

    nc = bacc.Bacc("TRN2", target_bir_lowering=False, debug=False)

    xT_d = nc.dram_tensor("xT", [D, B, S], bf16, kind="ExternalInput")
    wqkv_d = nc.dram_tensor("wqkvT", [D, DQ + DKV], bf16, kind="ExternalInput")
    wo_d = nc.dram_tensor("woT", [DQ, D], bf16, kind="ExternalInput")
    kaug_d = nc.dram_tensor("kaug_ext", [2, S], f32, kind="ExternalInput")
    qaug_d = nc.dram_tensor("qaug_ext", [HLOC, 2, S], f32, kind="ExternalInput")
    ident_d = nc.dram_tensor("ident", [64, 64], bf16, kind="ExternalInput")
    if causal:
        # mask pattern applied on the PE: qk += ident128.T @ mpat
        ident128_d = nc.dram_tensor("ident128", [128, 128], bf16,
                                    kind="ExternalInput")
        mpat_d = nc.dram_tensor("maskpat", [128, 128], bf16,
                                kind="ExternalInput")
    if generic_mask:
        maskT_d = nc.dram_tensor("maskT", [S, S], f32, kind="ExternalInput")
    out_d = nc.dram_tensor("out", [B, S, D], bf16, kind="ExternalOutput")

    def live(nt, mc):
        """is logitsT tile (keys nt*128.., queries mc*MC..) not fully masked"""
        if not causal:
            return True
        return nt * 128 <= mc * MC + MC - 1

    def crossing(nt, mc):
        """does the tile cross the causal diagonal (needs mask pattern)"""
        if not causal:
            return False
        return live(nt, mc) and nt * 128 + 127 > mc * MC

    with tile.TileContext(nc) as tc:
        with tc.tile_pool(name="res", bufs=1) as res, \
             tc.tile_pool(name="dbl", bufs=2) as dbl, \
             tc.tile_pool(name="sbp", bufs=3) as sbp, \
             tc.tile_pool(name="ps", bufs=1, space="PSUM") as ps:

            # ---- resident weights ----------------------------------------
            # wqkv quarters go on the SP queue (needed by the first matmul);
            # everything else loads via the ACT queue so the first xt DMA
            # isn't stuck behind resident loads on the in-order SP queue.
            wqkv_sb = res.tile([128, NKT, DQ + DKV], bf16, tag="wqkv")
            qtr = NKT // 4

            def _wqkv_quarter(qi):
                sl = slice(qi * qtr * 128, (qi + 1) * qtr * 128)
                nc.sync.dma_start(
                    wqkv_sb[:, qi * qtr:(qi + 1) * qtr, :],
                    wqkv_d.ap()[sl, :]
                    .rearrange("(kt p) q -> p kt q", p=128))

            # quarter 0 now; 1-3 deferred until after the first xt DMA so the
            # first projection matmul isn't stuck behind them on DMA_ENGINES
            _wqkv_quarter(0)
            deferred = [lambda qi=qi: _wqkv_quarter(qi) for qi in range(1, 4)]
            wo_sb = res.tile([128, NHP, D], bf16, tag="wo")
            ident_sb = res.tile([64, 64], bf16, tag="ident")
            if causal:
                ident128_sb = res.tile([128, 128], bf16, tag="ident128")
                mpat_sb = res.tile([128, 128], bf16, tag="mpat")

            def _load_misc():
                nc.scalar.dma_start(
                    wo_sb[:],
                    wo_d.ap()[:].rearrange("(hp p) e -> p hp e", p=128))
                nc.scalar.dma_start(ident_sb[:], ident_d.ap()[:])
                if causal:
                    nc.scalar.dma_start(ident128_sb[:], ident128_d.ap()[:])
                    nc.scalar.dma_start(mpat_sb[:], mpat_d.ap()[:])

            deferred.append(lambda: _load_misc())

            # per-b double-buffered activations (allocated inside the b loop)
            state = {}
            alt = {"i": 0}  # DVE/Pool alternation for mask adds + oproj evicts

            def proj_mc(b, mc):
                """projections for m-chunk mc of batch b"""
                kaug, qaug, vt = state["kaug"], state["qaug"], state["vt"]
                mco = mc * MC
                qp = [ps.tile([128, MC], f32, tag="qk", bufs=4,
                              name=f"qp{hp}") for hp in range(NHP)]
                kvp = ps.tile([128, MC], f32, tag="ps4", bufs=4, name="kvp")
                KQ = 4  # k-tiles per xt DMA
                for ktq in range(NKT // KQ):
                    xt = sbp.tile([128, KQ, MC], bf16, tag="xt", bufs=6)
                    nc.sync.dma_start(
                        xt[:], xT_d.ap()[ktq * KQ * 128:(ktq + 1) * KQ * 128,
                                         b, mco:mco + MC]
                        .rearrange("(k p) m -> p k m", p=128))
                    while deferred:
                        deferred.pop(0)()
                    for kq in range(KQ):
                        kt = ktq * KQ + kq
                        st, sp = (kt == 0), (kt == NKT - 1)
                        for hp in range(NHP):
                            nc.tensor.matmul(
                                qp[hp][:],
                                wqkv_sb[:, kt, hp * 128:(hp + 1) * 128],
                                xt[:, kq], start=st, stop=sp)
                        nc.tensor.matmul(kvp[:], wqkv_sb[:, kt, DQ:DQ + DKV],
                                         xt[:, kq], start=st, stop=sp)
                # evictions, spread across DVE/ACT so qp frees fast
                # (GPSIMD cannot access PSUM)
                for hp in range(NHP):
                    # even head of the pair: psum rows 0:64 -> qaug rows 0:64
                    nc.vector.tensor_copy(qaug[2 * hp][0:64, mco:mco + MC],
                                          qp[hp][0:64, :])
                    # odd head: rows 64:128, engine-copy then DMA shift
                    qtmp = sbp.tile([128, MC], f32r, tag="qtmp", bufs=4,
                                    name="qtmp")
                    nc.vector.tensor_copy(qtmp[64:128, :], qp[hp][64:128, :])
                    nc.sync.dma_start(qaug[2 * hp + 1][0:64, mco:mco + MC],
                                      qtmp[64:128, :])
                nc.vector.tensor_copy(kaug[0:64, mco:mco + MC], kvp[0:64, :])
                vtmp = sbp.tile([128, MC], bf16, tag="vtmp", bufs=2,
                                name="vtmp")
                nc.scalar.activation(vtmp[64:128, :], kvp[64:128, :],
                                     mybir.ActivationFunctionType.Copy)
                nc.sync.dma_start(vt[0:64, mco:mco + MC], vtmp[64:128, :])

            def vtrans(b):
                """transpose vT -> v (vaug), groups of 8 n-tiles per psum"""
                vt, vaug = state["vt"], state["vaug"]
                for g in range((NNT + 7) // 8):
                    nts = range(g * 8, min((g + 1) * 8, NNT))
                    vtp = ps.tile([128, 512], bf16, tag="ps4", bufs=4,
                                  name="vtp")
                    for j, nt in enumerate(nts):
                        nc.tensor.transpose(
                            vtp[:, j * 64:(j + 1) * 64],
                            vt[0:64, nt * 128:(nt + 1) * 128], ident_sb[:])
                    nc.vector.tensor_copy(vaug[:, nts.start:nts.stop, 0:HD],
                                          vtp[:, 0:64 * len(nts)].rearrange(
                                              "p (t d) -> p t d", d=64))

            def attn_head(b, pair, h):
                kaug, qaug, vaug = state["kaug"], state["qaug"], state["vaug"]
                OT = state["OT"]
                hp, odd = h // 2, h % 2
                av = [ps.tile([128, MC], f32, tag="ps4", bufs=4,
                              name=f"av{c}") for c in range(2)]
                nlive = [nt for nt in range(NNT)
                         if live(nt, 2 * pair) or live(nt, 2 * pair + 1)]
                last_nt = nlive[-1]

                def emit_av(nt, c, pt):
                    st = (nt == 0)
                    if causal:
                        mc = 2 * pair + c
                        # columns whose diagonal (last) tile is nt
                        sl = max(0, nt * 128 - mc * MC)
                        sh = min(MC, nt * 128 + 128 - mc * MC)
                        if sh > sl:
                            nc.tensor.matmul(
                                av[c][0:HD + 1, sl:sh],
                                vaug[:, nt, :], pt[:, sl:sh],
                                start=st, stop=True,
                                skip_group_check=True)
                            if sh < MC:
                                nc.tensor.matmul(
                                    av[c][0:HD + 1, sh:MC],
                                    vaug[:, nt, :], pt[:, sh:MC],
                                    start=st, stop=False,
                                    skip_group_check=True)
                        else:
                            nc.tensor.matmul(
                                av[c][0:HD + 1, :], vaug[:, nt, :], pt[:],
                                start=st, stop=False,
                                skip_group_check=True)
                    else:
                        nc.tensor.matmul(
                            av[c][0:HD + 1, :], vaug[:, nt, :], pt[:],
                            start=st, stop=(nt == last_nt))

                # software pipeline: AV runs LAG chunk-steps behind
                # logits/exp so the exp round-trip latency never blocks the
                # in-order PE queue
                LAG = 4
                fifo = []
                for nt in nlive:
                    for c in range(2):
                        mc = 2 * pair + c
                        if not live(nt, mc):
                            continue
                        o = max(0, nt * 128 - mc * MC) if causal else 0
                        qk = ps.tile([128, MC], f32, tag="qk", bufs=4,
                                     name="qk")
                        pt = sbp.tile([128, MC], bf16, tag="pt", bufs=6,
                                      name="pt")
                        cross = crossing(nt, mc)
                        nc.tensor.matmul(
                            qk[:, o:MC],
                            kaug[:, nt * 128:(nt + 1) * 128],
                            qaug[h][:, mc * MC + o:(mc + 1) * MC],
                            start=True, stop=not cross,
                            skip_group_check=cross)
                        if generic_mask:
                            mtile = sbp.tile([128, MC], f32, tag="mt",
                                             name="mt")
                            nc.sync.dma_start(
                                mtile[:],
                                maskT_d.ap()[nt * 128:(nt + 1) * 128,
                                             mc * MC:(mc + 1) * MC])
                            nc.vector.tensor_add(qk[:], qk[:], mtile[:])
                        elif cross:
                            # accumulate the causal pattern on the PE
                            nc.tensor.matmul(
                                qk[:, o:o + 128], ident128_sb[:],
                                mpat_sb[:], start=False, stop=True,
                                skip_group_check=True)
                        nc.scalar.activation(
                            pt[:, o:MC], qk[:, o:MC],
                            mybir.ActivationFunctionType.Exp)
                        fifo.append((nt, c, pt))
                        if len(fifo) > LAG:
                            emit_av(*fifo.pop(0))
                for item in fifo:
                    emit_av(*item)
                # normalize per chunk (chunk 0's chain overlaps chunk 1's
                # remaining AV matmuls): denom row -> broadcast -> divide
                # folded into the bf16 psum eviction
                qdiv = None
                if odd:
                    qdiv = sbp.tile([64, MPAIR], bf16, tag="qdiv", bufs=2,
                                    name="qdiv")
                for c in range(2):
                    cs = slice(c * MC, (c + 1) * MC)
                    srow = sbp.tile([1, MC], f32, tag="srow", bufs=4,
                                    name="srow")
                    nc.vector.tensor_copy(srow[0:1, :], av[c][HD:HD + 1, :])
                    nc.vector.reciprocal(srow[0:1, :], srow[0:1, :])
                    rbc = sbp.tile([128, MC], f32, tag="rbc", bufs=4,
                                   name="rbc")
                    nc.gpsimd.partition_broadcast(rbc[:], srow[0:1, :])
                    dst = qdiv[0:64, cs] if odd else OT[0:64, hp, cs]
                    nc.vector.tensor_mul(dst, av[c][0:64, :], rbc[0:64, :])
                if odd:
                    nc.sync.dma_start(OT[64:128, hp, :], qdiv[0:64, :])

            def oproj_mt(b, pair, OT, mtl):
                """out-projection for one 128-query tile of a pair"""
                mt = pair * (MPAIR // 128) + mtl
                ob = sbp.tile([128, D], bf16, tag="ob", bufs=2, name="ob")
                for ec in range(NEC):
                    op = ps.tile([128, MC], f32, tag="ps4", bufs=4, name="op")
                    for hp in range(NHP):
                        nc.tensor.matmul(
                            op[:],
                            OT[:, hp, mtl * 128:(mtl + 1) * 128],
                            wo_sb[:, hp, ec * MC:(ec + 1) * MC],
                            start=(hp == 0), stop=(hp == NHP - 1))
                    nc.vector.tensor_copy(ob[:, ec * MC:(ec + 1) * MC],
                                          op[:])
                nc.sync.dma_start(
                    out_d.ap()[b, mt * 128:(mt + 1) * 128, :], ob[:])

            # ---- schedule: proj/attention with interleaved out-proj ------
            pending = []

            def drain(k):
                for _ in range(min(k, len(pending))):
                    pending.pop(0)()

            for _rep in range(cfg.get("reps", 1)):
                for b in range(B):
                    state["kaug"] = dbl.tile([66, S], f32r, tag="kaug",
                                             bufs=2, name="kaug")
                    nc.scalar.dma_start(state["kaug"][64:66, :],
                                        kaug_d.ap()[:].bitcast(f32r))
                    qaug = []
                    for h in range(HLOC):
                        t = dbl.tile([66, S], f32r, tag=f"qaug{h}", bufs=2,
                                     name=f"qaug{h}")
                        nc.scalar.dma_start(t[64:66, :],
                                            qaug_d.ap()[h].bitcast(f32r))
                        qaug.append(t)
                    state["qaug"] = qaug
                    state["vt"] = dbl.tile([64, S], bf16, tag="vt", bufs=2,
                                           name="vt")
                    state["vaug"] = dbl.tile([128, NNT, HD + 1], bf16,
                                             tag="vaug", bufs=2, name="vaug")
                    nc.vector.memset(state["vaug"][:], 1.0)

                    for mc in range(NMC):
                        proj_mc(b, mc)
                        drain(2)
                    vtrans(b)
                    drain(2)
                    for pair in range(NPAIR):
                        OT = dbl.tile([128, NHP, MPAIR], bf16, tag="OT",
                                      bufs=3, name="OT")
                        state["OT"] = OT
                        for h in range(HLOC):
                            attn_head(b, pair, h)
                            drain((0, 2, 3, 3)[h])
                        for mtl in range(MPAIR // 128):
                            pending.append(
                                lambda b=b, pair=pair, OT=OT, mtl=mtl:
                                oproj_mt(b, pair, OT, mtl))
                drain(len(pending))

    nc.compile()
    return nc


# ---------------------------------------------------------------------------
# host side
# ---------------------------------------------------------------------------

def _analyze_mask(mask2d, S):
    """classify mask; return (causal, zeros, n_lo, n_hi)"""
    masked = mask2d < -1e8
    if not masked.any():
        return False, True, np.zeros(S, np.int64), np.full(S, S - 1, np.int64)
    tri = np.triu(np.ones((S, S), bool), 1)
    if (masked == tri).all() and (mask2d[~masked] == 0).all():
        return True, False, np.zeros(S, np.int64), np.arange(S)
    allowed = ~masked
    # guard fully-masked rows (keep index 0; softmax row is garbage anyway)
    any_allowed = allowed.any(axis=1)
    idx = np.arange(S)[None, :]
    n_hi = np.where(any_allowed, np.where(allowed, idx, -1).max(axis=1), 0)
    n_lo = np.where(any_allowed, np.where(allowed, idx, S).min(axis=1), 0)
    return False, False, n_lo, n_hi


def _bf16(a):
    import ml_dtypes
    return np.ascontiguousarray(a).astype(ml_dtypes.bfloat16)


def _make_inputs_for_core(core, x, wq, wk, wv, wo, slopes, mask, cfg):
    B, S, D, HLOC, HD = cfg["B"], cfg["S"], cfg["D"], cfg["HLOC"], cfg["HD"]
    h0 = core * HLOC
    kv = core  # one kv head per core
    scale = 1.0 / np.sqrt(HD)

    xT = np.ascontiguousarray(x.transpose(2, 0, 1))                 # [D,B,S]
    wqkvT = np.concatenate(
        [wq[h0 * HD:(h0 + HLOC) * HD] * scale,
         wk[kv * HD:(kv + 1) * HD],
         wv[kv * HD:(kv + 1) * HD]], axis=0).T                      # [D,384]
    woT = np.ascontiguousarray(wo[:, h0 * HD:(h0 + HLOC) * HD].T)   # [DQ,D]

    n = np.arange(S, dtype=np.float32)
    kaug_ext = np.stack([n, np.ones(S, np.float32)])                # [2,S]

    qaug_ext = np.zeros((HLOC, 2, S), np.float32)
    for i in range(HLOC):
        sl = float(slopes[h0 + i])
        # stabilizer c[m] = max over allowed n of slope*(n-m), clipped >= 0
        c = np.maximum(0.0, np.maximum(sl * (cfg["n_hi"] - n),
                                       sl * (cfg["n_lo"] - n)))
        qaug_ext[i, 0, :] = sl
        qaug_ext[i, 1, :] = -sl * n - c

    ident = np.eye(64, dtype=np.float32)

    ins = {"xT": _bf16(xT), "wqkvT": _bf16(wqkvT), "woT": _bf16(woT),
           "kaug_ext": kaug_ext, "qaug_ext": qaug_ext,
           "ident": _bf16(ident)}
    if cfg["causal"]:
        ii = np.arange(128)[:, None]
        jj = np.arange(128)[None, :]
        ins["maskpat"] = _bf16(np.where(ii > jj, NEG, 0.0))
        ins["ident128"] = _bf16(np.eye(128))
    if cfg["generic_mask"]:
        ins["maskT"] = np.ascontiguousarray(mask[0, 0].T)
    return ins


def kernel(x, wq, wk, wv, wo, slopes, mask):
    from concourse.bass_utils import run_bass_kernel_spmd

    x = np.asarray(x, dtype=np.float32)
    wq = np.asarray(wq, dtype=np.float32)
    wk = np.asarray(wk, dtype=np.float32)
    wv = np.asarray(wv, dtype=np.float32)
    wo = np.asarray(wo, dtype=np.float32)
    slopes = np.asarray(slopes, dtype=np.float32)
    mask = np.asarray(mask, dtype=np.float32)

    B, S, D = x.shape
    HQ = 32
    HD = D // HQ
    n_cores = 8
    HLOC = HQ // n_cores

    causal, zeros, n_lo, n_hi = _analyze_mask(mask[0, 0], S)
    cfg = dict(B=B, S=S, D=D, HLOC=HLOC, HD=HD, MC=512,
               causal=causal, generic_mask=not (causal or zeros),
               n_lo=n_lo, n_hi=n_hi)

    nc = build_program(cfg)
    in_maps = [_make_inputs_for_core(c, x, wq, wk, wv, wo, slopes, mask, cfg)
               for c in range(n_cores)]
    res = run_bass_kernel_spmd(nc, in_maps, core_ids=list(range(n_cores)))
    out = np.zeros((B, S, D), np.float32)
    for c in range(n_cores):
        out += res.results[c]["out"].astype(np.float32)
    return out


if __name__ == "__main__":
    pass


# revision 39
# speedup vs baseline: 1.7105x; 1.0656x over previous
"""GQA attention kernel for 8 TRN2 NeuronCores (tensor-parallel over heads).

Problem: B=2, S=2048, D=2048, HQ=32, HKV=8, HD=64, ALiBi + additive mask,
softmax, out-projection.  Each core owns 4 q-heads (= 1 kv head); each core
computes a full-shape partial of the output (its heads' contribution through
wo), and the host sums the 8 partials.

v2 layout strategy (per core):
  - data path in bf16 (x, wq/wk/wv, wo, v, exp(logits), attention outputs,
    DRAM output partial); psum stays f32.  ALiBi aug rows need f32 range
    (slope*m up to ~2e3), so the logits matmul runs f32r on f32 qaug/kaug
    whose data rows are written from the f32 projection psum.
  - logits computed TRANSPOSED: logitsT[n, m] = kaug.T @ qaug with the
    contraction dim augmented by 2 rows that add alibi slope*(n-m) and a
    per-query stabilizer -c[m] for free:
       kaug = [kT(64); n; 1]            (shared by all 4 heads)
       qaug_h = [qT_h(64); slope_h; -slope_h*m - c_h[m]]
  - PT = exp(logitsT) in bf16; AV matmul uses vaug = [v | ones] so the ones
    column accumulates softmax denominators in psum row 64.  AV matmuls are
    column-trimmed to the causal region with per-diagonal-block stop flags.
  - normalization: denominator row is copied out of psum, partition-broadcast
    (Pool), and divided into the AV psum during the bf16 eviction (DVE).
    Odd heads are DMA-shifted to partitions 64:127 so the o-projection reads
    one contiguous [128, m] stationary per head pair.
  - out-projection is split into per-128-query units and software-pipelined:
    units are interleaved into the NEXT attention/projection phase so the PE
    never waits on the normalize chain.
  - causal masks: dead logit tiles are skipped; diagonal-crossing tiles get a
    precomputed [128,128] additive pattern (DVE/Pool alternating).
"""

import sys

sys.path.insert(0, "/opt/trn_rl_repo")

import numpy as np

NEG = -1e9


# ---------------------------------------------------------------------------
# device program builder
# ---------------------------------------------------------------------------

def build_program(cfg):
    import concourse.bass as bass  # noqa: F401
    import concourse.mybir as mybir
    import concourse.tile as tile
    from concourse import bacc

    f32 = mybir.dt.float32
    f32r = mybir.dt.float32r
    bf16 = mybir.dt.bfloat16

    B, S, D = cfg["B"], cfg["S"], cfg["D"]
    HLOC, HD = cfg["HLOC"], cfg["HD"]
    MC = cfg["MC"]                    # m-chunk (<= 512, psum bank)
    MPAIR = 2 * MC                    # exp / AV / normalize granularity
    causal = cfg["causal"]
    generic_mask = cfg["generic_mask"]

    DQ = HLOC * HD                    # local q dims (256)
    DKV = 2 * HD                      # local kv dims (128)
    NKT = D // 128                    # contraction k-tiles for projections
    NNT = S // 128                    # n-tiles (keys)
    NMC = S // MC                     # m-chunks per b
    NPAIR = S // MPAIR                # m-pairs per b
    NHP = HLOC // 2                   # head pairs
    NEC = D // MC                     # out-proj e-chunks

    nc = bacc.Bacc("TRN2", target_bir_lowering=False, debug=False)

    fp8 = mybir.dt.float8e4
    WPAD = 512                        # wqkv cols padded for 512B dma elems
    # x and wqkv as fp8 (hi, lo) residual pairs for DoubleRow matmuls;
    # layouts [kt, slot, p, cols]; w slots (0=lo, 1=hi), x slots (0=hi, 1=lo)
    xT_d = nc.dram_tensor("xT2", [NKT, 2, 128, B, S], fp8,
                          kind="ExternalInput")
    wqkv_d = nc.dram_tensor("wqkv2", [NKT, 2, 128, WPAD], fp8,
                            kind="ExternalInput")
    wo_d = nc.dram_tensor("woT", [DQ, D], bf16, kind="ExternalInput")
    kaug_d = nc.dram_tensor("kaug_ext", [2, S], f32, kind="ExternalInput")
    qaug_d = nc.dram_tensor("qaug_ext", [HLOC, 2, S], f32, kind="ExternalInput")
    ident_d = nc.dram_tensor("ident", [64, 64], bf16, kind="ExternalInput")
    if causal:
        # mask pattern applied on the PE: qk += ident128.T @ mpat
        ident128_d = nc.dram_tensor("ident128", [128, 128], bf16,
                                    kind="ExternalInput")
        mpat_d = nc.dram_tensor("maskpat", [128, 128], bf16,
                                kind="ExternalInput")
    if generic_mask:
        maskT_d = nc.dram_tensor("maskT", [S, S], f32, kind="ExternalInput")
    out_d = nc.dram_tensor("out", [B, S, D], bf16, kind="ExternalOutput")

    def live(nt, mc):
        """is logitsT tile (keys nt*128.., queries mc*MC..) not fully masked"""
        if not causal:
            return True
        return nt * 128 <= mc * MC + MC - 1

    def crossing(nt, mc):
        """does the tile cross the causal diagonal (needs mask pattern)"""
        if not causal:
            return False
        return live(nt, mc) and nt * 128 + 127 > mc * MC

    with tile.TileContext(nc) as tc:
        with tc.tile_pool(name="res", bufs=1) as res, \
             tc.tile_pool(name="dbl", bufs=2) as dbl, \
             tc.tile_pool(name="sbp", bufs=3) as sbp, \
             tc.tile_pool(name="ps", bufs=1, space="PSUM") as ps:

            # ---- resident weights ----------------------------------------
            # wqkv quarters go on the SP queue (needed by the first matmul);
            # everything else loads via the ACT queue so the first xt DMA
            # isn't stuck behind resident loads on the in-order SP queue.
            wqkv_sb = res.tile([128, NKT, 2, WPAD], fp8, tag="wqkv")
            qtr = NKT // 4

            def _wqkv_quarter(qi):
                nc.sync.dma_start(
                    wqkv_sb[:, qi * qtr:(qi + 1) * qtr, :, :],
                    wqkv_d.ap()[qi * qtr:(qi + 1) * qtr]
                    .rearrange("kt two p q -> p kt two q"))

            # quarter 0 now; 1-3 deferred until after the first xt DMA so the
            # first projection matmul isn't stuck behind them on DMA_ENGINES
            _wqkv_quarter(0)
            deferred = [lambda qi=qi: _wqkv_quarter(qi) for qi in range(1, 4)]
            wo_sb = res.tile([128, NHP, D], bf16, tag="wo")
            ident_sb = res.tile([64, 64], bf16, tag="ident")
            if causal:
                ident128_sb = res.tile([128, 128], bf16, tag="ident128")
                mpat_sb = res.tile([128, 128], bf16, tag="mpat")

            def _load_misc():
                nc.scalar.dma_start(
                    wo_sb[:],
                    wo_d.ap()[:].rearrange("(hp p) e -> p hp e", p=128))
                nc.scalar.dma_start(ident_sb[:], ident_d.ap()[:])
                if causal:
                    nc.scalar.dma_start(ident128_sb[:], ident128_d.ap()[:])
                    nc.scalar.dma_start(mpat_sb[:], mpat_d.ap()[:])

            deferred.append(lambda: _load_misc())

            # per-b double-buffered activations (allocated inside the b loop)
            state = {}
            alt = {"i": 0}  # DVE/Pool alternation for mask adds + oproj evicts

            def proj_mc(b, mc):
                """projections for m-chunk mc of batch b"""
                kaug, qaug, vt = state["kaug"], state["qaug"], state["vt"]
                mco = mc * MC
                qp = [ps.tile([128, MC], f32, tag="qk", bufs=4,
                              name=f"qp{hp}") for hp in range(NHP)]
                kvp = ps.tile([128, MC], f32, tag="ps4", bufs=4, name="kvp")
                KQ = 4  # k-tiles per xt DMA
                DR = mybir.MatmulPerfMode.DoubleRow
                for ktq in range(NKT // KQ):
                    xt = sbp.tile([128, KQ, 2, MC], fp8, tag="xt", bufs=6)
                    nc.sync.dma_start(
                        xt[:], xT_d.ap()[ktq * KQ:(ktq + 1) * KQ, :, :,
                                         b, mco:mco + MC]
                        .rearrange("kt two p m -> p kt two m"))
                    while deferred:
                        deferred.pop(0)()
                    st = (ktq == 0)
                    sp = (ktq == NKT // KQ - 1)
                    groups = [(qp[0], 0), (qp[1], 128), (kvp, DQ)]
                    for dst, g0 in groups:
                        csl = slice(g0, g0 + 128) if g0 < DQ                             else slice(DQ, DQ + DKV)
                        # hi*hi over kt pairs
                        for kp in range(KQ // 2):
                            nc.tensor.matmul(
                                dst[:],
                                wqkv_sb[:, ktq * KQ + 2 * kp:
                                        ktq * KQ + 2 * kp + 2, 1, csl],
                                xt[:, 2 * kp:2 * kp + 2, 0, :],
                                start=st and kp == 0, stop=False,
                                perf_mode=DR)
                        # cross terms (w_lo x_hi + w_hi x_lo) per kt
                        for kq in range(KQ):
                            nc.tensor.matmul(
                                dst[:],
                                wqkv_sb[:, ktq * KQ + kq, :, csl],
                                xt[:, kq, :, :],
                                start=False, stop=sp and kq == KQ - 1,
                                perf_mode=DR)
                # evictions, spread across DVE/ACT so qp frees fast
                # (GPSIMD cannot access PSUM)
                WS = 1.0 / 1024.0  # undo the fp8 weight scaling
                for hp in range(NHP):
                    # even head of the pair: psum rows 0:64 -> qaug rows 0:64
                    nc.vector.tensor_scalar_mul(
                        qaug[2 * hp][0:64, mco:mco + MC], qp[hp][0:64, :], WS)
                    # odd head: rows 64:128, engine-copy then DMA shift
                    qtmp = sbp.tile([128, MC], f32r, tag="qtmp", bufs=4,
                                    name="qtmp")
                    nc.vector.tensor_scalar_mul(qtmp[64:128, :],
                                                qp[hp][64:128, :], WS)
                    nc.sync.dma_start(qaug[2 * hp + 1][0:64, mco:mco + MC],
                                      qtmp[64:128, :])
                nc.vector.tensor_scalar_mul(kaug[0:64, mco:mco + MC],
                                            kvp[0:64, :], WS)
                vtmp = sbp.tile([128, MC], bf16, tag="vtmp", bufs=2,
                                name="vtmp")
                nc.scalar.activation(vtmp[64:128, :], kvp[64:128, :],
                                     mybir.ActivationFunctionType.Copy,
                                     scale=WS)
                nc.sync.dma_start(vt[0:64, mco:mco + MC], vtmp[64:128, :])

            def vtrans(b):
                """transpose vT -> v (vaug), groups of 8 n-tiles per psum"""
                vt, vaug = state["vt"], state["vaug"]
                for g in range((NNT + 7) // 8):
                    nts = range(g * 8, min((g + 1) * 8, NNT))
                    vtp = ps.tile([128, 512], bf16, tag="ps4", bufs=4,
                                  name="vtp")
                    for j, nt in enumerate(nts):
                        nc.tensor.transpose(
                            vtp[:, j * 64:(j + 1) * 64],
                            vt[0:64, nt * 128:(nt + 1) * 128], ident_sb[:])
                    nc.vector.tensor_copy(vaug[:, nts.start:nts.stop, 0:HD],
                                          vtp[:, 0:64 * len(nts)].rearrange(
                                              "p (t d) -> p t d", d=64))

            def attn_head(b, pair, h):
                kaug, qaug, vaug = state["kaug"], state["qaug"], state["vaug"]
                OT = state["OT"]
                hp, odd = h // 2, h % 2
                av = [ps.tile([128, MC], f32, tag="ps4", bufs=4,
                              name=f"av{c}") for c in range(2)]
                nlive = [nt for nt in range(NNT)
                         if live(nt, 2 * pair) or live(nt, 2 * pair + 1)]
                last_nt = nlive[-1]

                def emit_av(nt, c, pt):
                    st = (nt == 0)
                    if causal:
                        mc = 2 * pair + c
                        # columns whose diagonal (last) tile is nt
                        sl = max(0, nt * 128 - mc * MC)
                        sh = min(MC, nt * 128 + 128 - mc * MC)
                        if sh > sl:
                            nc.tensor.matmul(
                                av[c][0:HD + 1, sl:sh],
                                vaug[:, nt, :], pt[:, sl:sh],
                                start=st, stop=True,
                                skip_group_check=True)
                            if sh < MC:
                                nc.tensor.matmul(
                                    av[c][0:HD + 1, sh:MC],
                                    vaug[:, nt, :], pt[:, sh:MC],
                                    start=st, stop=False,
                                    skip_group_check=True)
                        else:
                            nc.tensor.matmul(
                                av[c][0:HD + 1, :], vaug[:, nt, :], pt[:],
                                start=st, stop=False,
                                skip_group_check=True)
                    else:
                        nc.tensor.matmul(
                            av[c][0:HD + 1, :], vaug[:, nt, :], pt[:],
                            start=st, stop=(nt == last_nt))

                # software pipeline: AV runs LAG chunk-steps behind
                # logits/exp so the exp round-trip latency never blocks the
                # in-order PE queue
                LAG = 4
                fifo = []
                for nt in nlive:
                    for c in range(2):
                        mc = 2 * pair + c
                        if not live(nt, mc):
                            continue
                        o = max(0, nt * 128 - mc * MC) if causal else 0
                        qk = ps.tile([128, MC], f32, tag="qk", bufs=4,
                                     name="qk")
                        pt = sbp.tile([128, MC], bf16, tag="pt", bufs=6,
                                      name="pt")
                        cross = crossing(nt, mc)
                        nc.tensor.matmul(
                            qk[:, o:MC],
                            kaug[:, nt * 128:(nt + 1) * 128],
                            qaug[h][:, mc * MC + o:(mc + 1) * MC],
                            start=True, stop=not cross,
                            skip_group_check=cross)
                        if generic_mask:
                            mtile = sbp.tile([128, MC], f32, tag="mt",
                                             name="mt")
                            nc.sync.dma_start(
                                mtile[:],
                                maskT_d.ap()[nt * 128:(nt + 1) * 128,
                                             mc * MC:(mc + 1) * MC])
                            nc.vector.tensor_add(qk[:], qk[:], mtile[:])
                        elif cross:
                            # accumulate the causal pattern on the PE
                            nc.tensor.matmul(
                                qk[:, o:o + 128], ident128_sb[:],
                                mpat_sb[:], start=False, stop=True,
                                skip_group_check=True)
                        nc.scalar.activation(
                            pt[:, o:MC], qk[:, o:MC],
                            mybir.ActivationFunctionType.Exp)
                        fifo.append((nt, c, pt))
                        if len(fifo) > LAG:
                            emit_av(*fifo.pop(0))
                for item in fifo:
                    emit_av(*item)
                # normalize per chunk (chunk 0's chain overlaps chunk 1's
                # remaining AV matmuls): denom row -> broadcast -> divide
                # folded into the bf16 psum eviction
                qdiv = None
                if odd:
                    qdiv = sbp.tile([64, MPAIR], bf16, tag="qdiv", bufs=2,
                                    name="qdiv")
                for c in range(2):
                    cs = slice(c * MC, (c + 1) * MC)
                    srow = sbp.tile([1, MC], f32, tag="srow", bufs=4,
                                    name="srow")
                    nc.vector.tensor_copy(srow[0:1, :], av[c][HD:HD + 1, :])
                    nc.vector.reciprocal(srow[0:1, :], srow[0:1, :])
                    rbc = sbp.tile([128, MC], f32, tag="rbc", bufs=4,
                                   name="rbc")
                    nc.gpsimd.partition_broadcast(rbc[:], srow[0:1, :])
                    dst = qdiv[0:64, cs] if odd else OT[0:64, hp, cs]
                    nc.vector.tensor_mul(dst, av[c][0:64, :], rbc[0:64, :])
                if odd:
                    nc.sync.dma_start(OT[64:128, hp, :], qdiv[0:64, :])

            def oproj_mt(b, pair, OT, mtl):
                """out-projection for one 128-query tile of a pair"""
                mt = pair * (MPAIR // 128) + mtl
                ob = sbp.tile([128, D], bf16, tag="ob", bufs=2, name="ob")
                for ec in range(NEC):
                    op = ps.tile([128, MC], f32, tag="ps4", bufs=4, name="op")
                    for hp in range(NHP):
                        nc.tensor.matmul(
                            op[:],
                            OT[:, hp, mtl * 128:(mtl + 1) * 128],
                            wo_sb[:, hp, ec * MC:(ec + 1) * MC],
                            start=(hp == 0), stop=(hp == NHP - 1))
                    nc.vector.tensor_copy(ob[:, ec * MC:(ec + 1) * MC],
                                          op[:])
                nc.sync.dma_start(
                    out_d.ap()[b, mt * 128:(mt + 1) * 128, :], ob[:])

            # ---- schedule: proj/attention with interleaved out-proj ------
            pending = []

            def drain(k):
                for _ in range(min(k, len(pending))):
                    pending.pop(0)()

            for _rep in range(cfg.get("reps", 1)):
                for b in range(B):
                    state["kaug"] = dbl.tile([66, S], f32r, tag="kaug",
                                             bufs=2, name="kaug")
                    nc.scalar.dma_start(state["kaug"][64:66, :],
                                        kaug_d.ap()[:].bitcast(f32r))
                    qaug = []
                    for h in range(HLOC):
                        t = dbl.tile([66, S], f32r, tag=f"qaug{h}", bufs=2,
                                     name=f"qaug{h}")
                        nc.scalar.dma_start(t[64:66, :],
                                            qaug_d.ap()[h].bitcast(f32r))
                        qaug.append(t)
                    state["qaug"] = qaug
                    state["vt"] = dbl.tile([64, S], bf16, tag="vt", bufs=2,
                                           name="vt")
                    state["vaug"] = dbl.tile([128, NNT, HD + 1], bf16,
                                             tag="vaug", bufs=2, name="vaug")
                    nc.vector.memset(state["vaug"][:], 1.0)

                    for mc in range(NMC):
                        proj_mc(b, mc)
                        drain(2)
                    vtrans(b)
                    drain(2)
                    for pair in range(NPAIR):
                        OT = dbl.tile([128, NHP, MPAIR], bf16, tag="OT",
                                      bufs=3, name="OT")
                        state["OT"] = OT
                        for h in range(HLOC):
                            attn_head(b, pair, h)
                            drain((0, 2, 3, 3)[h])
                        for mtl in range(MPAIR // 128):
                            pending.append(
                                lambda b=b, pair=pair, OT=OT, mtl=mtl:
                                oproj_mt(b, pair, OT, mtl))
                drain(len(pending))

    nc.compile()
    return nc


# ---------------------------------------------------------------------------
# host side
# ---------------------------------------------------------------------------

def _analyze_mask(mask2d, S):
    """classify mask; return (causal, zeros, n_lo, n_hi)"""
    masked = mask2d < -1e8
    if not masked.any():
        return False, True, np.zeros(S, np.int64), np.full(S, S - 1, np.int64)
    tri = np.triu(np.ones((S, S), bool), 1)
    if (masked == tri).all() and (mask2d[~masked] == 0).all():
        return True, False, np.zeros(S, np.int64), np.arange(S)
    allowed = ~masked
    # guard fully-masked rows (keep index 0; softmax row is garbage anyway)
    any_allowed = allowed.any(axis=1)
    idx = np.arange(S)[None, :]
    n_hi = np.where(any_allowed, np.where(allowed, idx, -1).max(axis=1), 0)
    n_lo = np.where(any_allowed, np.where(allowed, idx, S).min(axis=1), 0)
    return False, False, n_lo, n_hi


def _bf16(a):
    import ml_dtypes
    return np.ascontiguousarray(a).astype(ml_dtypes.bfloat16)


def _make_inputs_for_core(core, x, wq, wk, wv, wo, slopes, mask, cfg):
    B, S, D, HLOC, HD = cfg["B"], cfg["S"], cfg["D"], cfg["HLOC"], cfg["HD"]
    h0 = core * HLOC
    kv = core  # one kv head per core
    scale = 1.0 / np.sqrt(HD)

    import ml_dtypes
    FP8 = ml_dtypes.float8_e4m3
    NKT = D // 128
    DQ, DKV = HLOC * HD, 2 * HD
    WPAD = 512

    def _fp8_pair(a):
        hi = a.astype(FP8)
        lo = (a - hi.astype(np.float32)).astype(FP8)
        return hi, lo

    xT = np.ascontiguousarray(x.transpose(2, 0, 1))                 # [D,B,S]
    x_hi, x_lo = _fp8_pair(xT.reshape(NKT, 128, B, S))
    xT2 = np.stack([x_hi, x_lo], axis=1)                   # [kt,2,p,B,S]
    wqkvT = np.concatenate(
        [wq[h0 * HD:(h0 + HLOC) * HD] * scale,
         wk[kv * HD:(kv + 1) * HD],
         wv[kv * HD:(kv + 1) * HD]], axis=0).T                      # [D,384]
    wpad = np.zeros((D, WPAD), np.float32)
    wpad[:, :DQ + DKV] = wqkvT * 1024.0
    w_hi, w_lo = _fp8_pair(wpad.reshape(NKT, 128, WPAD))
    wqkv2 = np.stack([w_lo, w_hi], axis=1)                 # [kt,2,p,512]
    woT = np.ascontiguousarray(wo[:, h0 * HD:(h0 + HLOC) * HD].T)   # [DQ,D]

    n = np.arange(S, dtype=np.float32)
    kaug_ext = np.stack([n, np.ones(S, np.float32)])                # [2,S]

    qaug_ext = np.zeros((HLOC, 2, S), np.float32)
    for i in range(HLOC):
        sl = float(slopes[h0 + i])
        # stabilizer c[m] = max over allowed n of slope*(n-m), clipped >= 0
        c = np.maximum(0.0, np.maximum(sl * (cfg["n_hi"] - n),
                                       sl * (cfg["n_lo"] - n)))
        qaug_ext[i, 0, :] = sl
        qaug_ext[i, 1, :] = -sl * n - c

    ident = np.eye(64, dtype=np.float32)

    ins = {"xT2": xT2, "wqkv2": wqkv2, "woT": _bf16(woT),
           "kaug_ext": kaug_ext, "qaug_ext": qaug_ext,
           "ident": _bf16(ident)}
    if cfg["causal"]:
        ii = np.arange(128)[:, None]
        jj = np.arange(128)[None, :]
        ins["maskpat"] = _bf16(np.where(ii > jj, NEG, 0.0))
        ins["ident128"] = _bf16(np.eye(128))
    if cfg["generic_mask"]:
        ins["maskT"] = np.ascontiguousarray(mask[0, 0].T)
    return ins


def kernel(x, wq, wk, wv, wo, slopes, mask):
    from concourse.bass_utils import run_bass_kernel_spmd

    x = np.asarray(x, dtype=np.float32)
    wq = np.asarray(wq, dtype=np.float32)
    wk = np.asarray(wk, dtype=np.float32)
    wv = np.asarray(wv, dtype=np.float32)
    wo = np.asarray(wo, dtype=np.float32)
    slopes = np.asarray(slopes, dtype=np.float32)
    mask = np.asarray(mask, dtype=np.float32)

    B, S, D = x.shape
    HQ = 32
    HD = D // HQ
    n_cores = 8
    HLOC = HQ // n_cores

    causal, zeros, n_lo, n_hi = _analyze_mask(mask[0, 0], S)
    cfg = dict(B=B, S=S, D=D, HLOC=HLOC, HD=HD, MC=512,
               causal=causal, generic_mask=not (causal or zeros),
               n_lo=n_lo, n_hi=n_hi)

    nc = build_program(cfg)
    in_maps = [_make_inputs_for_core(c, x, wq, wk, wv, wo, slopes, mask, cfg)
               for c in range(n_cores)]
    res = run_bass_kernel_spmd(nc, in_maps, core_ids=list(range(n_cores)))
    out = np.zeros((B, S, D), np.float32)
    for c in range(n_cores):
        out += res.results[c]["out"].astype(np.float32)
    return out


if __name__ == "__main__":
    pass


# revision 48
# speedup vs baseline: 1.7136x; 1.0018x over previous
"""GQA attention kernel for 8 TRN2 NeuronCores (tensor-parallel over heads).

Problem: B=2, S=2048, D=2048, HQ=32, HKV=8, HD=64, ALiBi + additive mask,
softmax, out-projection.  Each core owns 4 q-heads (= 1 kv head); each core
computes a full-shape partial of the output (its heads' contribution through
wo), and the host sums the 8 partials.

v2 layout strategy (per core):
  - data path in bf16 (x, wq/wk/wv, wo, v, exp(logits), attention outputs,
    DRAM output partial); psum stays f32.  ALiBi aug rows need f32 range
    (slope*m up to ~2e3), so the logits matmul runs f32r on f32 qaug/kaug
    whose data rows are written from the f32 projection psum.
  - logits computed TRANSPOSED: logitsT[n, m] = kaug.T @ qaug with the
    contraction dim augmented by 2 rows that add alibi slope*(n-m) and a
    per-query stabilizer -c[m] for free:
       kaug = [kT(64); n; 1]            (shared by all 4 heads)
       qaug_h = [qT_h(64); slope_h; -slope_h*m - c_h[m]]
  - PT = exp(logitsT) in bf16; AV matmul uses vaug = [v | ones] so the ones
    column accumulates softmax denominators in psum row 64.  AV matmuls are
    column-trimmed to the causal region with per-diagonal-block stop flags.
  - normalization: denominator row is copied out of psum, partition-broadcast
    (Pool), and divided into the AV psum during the bf16 eviction (DVE).
    Odd heads are DMA-shifted to partitions 64:127 so the o-projection reads
    one contiguous [128, m] stationary per head pair.
  - out-projection is split into per-128-query units and software-pipelined:
    units are interleaved into the NEXT attention/projection phase so the PE
    never waits on the normalize chain.
  - causal masks: dead logit tiles are skipped; diagonal-crossing tiles get a
    precomputed [128,128] additive pattern (DVE/Pool alternating).
"""

import sys

sys.path.insert(0, "/opt/trn_rl_repo")

import numpy as np

NEG = -1e9


# ---------------------------------------------------------------------------
# device program builder
# ---------------------------------------------------------------------------

def build_program(cfg):
    import concourse.bass as bass  # noqa: F401
    import concourse.mybir as mybir
    import concourse.tile as tile
    from concourse import bacc

    f32 = mybir.dt.float32
    f32r = mybir.dt.float32r
    bf16 = mybir.dt.bfloat16

    B, S, D = cfg["B"], cfg["S"], cfg["D"]
    HLOC, HD = cfg["HLOC"], cfg["HD"]
    MC = cfg["MC"]                    # m-chunk (<= 512, psum bank)
    MPAIR = 2 * MC                    # exp / AV / normalize granularity
    causal = cfg["causal"]
    generic_mask = cfg["generic_mask"]

    DQ = HLOC * HD                    # local q dims (256)
    DKV = 2 * HD                      # local kv dims (128)
    NKT = D // 128                    # contraction k-tiles for projections
    NNT = S // 128                    # n-tiles (keys)
    NMC = S // MC                     # m-chunks per b
    NPAIR = S // MPAIR                # m-pairs per b
    NHP = HLOC // 2                   # head pairs
    NEC = D // MC                     # out-proj e-chunks

    nc = bacc.Bacc("TRN2", target_bir_lowering=False, debug=False)

    fp8 = mybir.dt.float8e4
    WPAD = 512                        # wqkv cols padded for 512B dma elems
    # x and wqkv as fp8 (hi, lo) residual pairs for DoubleRow matmuls;
    # layouts [kt, slot, p, cols]; w slots (0=lo, 1=hi), x slots (0=hi, 1=lo)
    xT_d = nc.dram_tensor("xT2", [NKT, 2, 128, B, S], fp8,
                          kind="ExternalInput")
    wqkv_d = nc.dram_tensor("wqkv2", [NKT, 2, 128, WPAD], fp8,
                            kind="ExternalInput")
    wo_d = nc.dram_tensor("woT", [DQ, D], bf16, kind="ExternalInput")
    kaug_d = nc.dram_tensor("kaug_ext", [2, S], f32, kind="ExternalInput")
    qaug_d = nc.dram_tensor("qaug_ext", [HLOC, 2, S], f32, kind="ExternalInput")
    ident_d = nc.dram_tensor("ident", [64, 64], bf16, kind="ExternalInput")
    if causal:
        # mask pattern applied on the PE: qk += ident128.T @ mpat
        ident128_d = nc.dram_tensor("ident128", [128, 128], bf16,
                                    kind="ExternalInput")
        mpat_d = nc.dram_tensor("maskpat", [128, 128], bf16,
                                kind="ExternalInput")
    if generic_mask:
        maskT_d = nc.dram_tensor("maskT", [S, S], f32, kind="ExternalInput")
    out_d = nc.dram_tensor("out", [B, S, D], bf16, kind="ExternalOutput")

    def live(nt, mc):
        """is logitsT tile (keys nt*128.., queries mc*MC..) not fully masked"""
        if not causal:
            return True
        return nt * 128 <= mc * MC + MC - 1

    def crossing(nt, mc):
        """does the tile cross the causal diagonal (needs mask pattern)"""
        if not causal:
            return False
        return live(nt, mc) and nt * 128 + 127 > mc * MC

    with tile.TileContext(nc) as tc:
        with tc.tile_pool(name="res", bufs=1) as res, \
             tc.tile_pool(name="dbl", bufs=2) as dbl, \
             tc.tile_pool(name="sbp", bufs=3) as sbp, \
             tc.tile_pool(name="ps", bufs=1, space="PSUM") as ps:

            # ---- resident weights ----------------------------------------
            # wqkv quarters go on the SP queue (needed by the first matmul);
            # everything else loads via the ACT queue so the first xt DMA
            # isn't stuck behind resident loads on the in-order SP queue.
            wqkv_sb = res.tile([128, NKT, 2, WPAD], fp8, tag="wqkv")
            qtr = NKT // 4

            def _wqkv_quarter(qi):
                nc.sync.dma_start(
                    wqkv_sb[:, qi * qtr:(qi + 1) * qtr, :, :],
                    wqkv_d.ap()[qi * qtr:(qi + 1) * qtr]
                    .rearrange("kt two p q -> p kt two q"))

            # quarter 0 now; 1-3 deferred until after the first xt DMA so the
            # first projection matmul isn't stuck behind them on DMA_ENGINES
            _wqkv_quarter(0)
            deferred = [lambda qi=qi: _wqkv_quarter(qi) for qi in range(1, 4)]
            wo_sb = res.tile([128, NHP, D], bf16, tag="wo")
            ident_sb = res.tile([64, 64], bf16, tag="ident")
            if causal:
                ident128_sb = res.tile([128, 128], bf16, tag="ident128")
                mpat_sb = res.tile([128, 128], bf16, tag="mpat")

            def _load_misc():
                nc.scalar.dma_start(
                    wo_sb[:],
                    wo_d.ap()[:].rearrange("(hp p) e -> p hp e", p=128))
                nc.scalar.dma_start(ident_sb[:], ident_d.ap()[:])
                if causal:
                    nc.scalar.dma_start(ident128_sb[:], ident128_d.ap()[:])
                    nc.scalar.dma_start(mpat_sb[:], mpat_d.ap()[:])

            deferred.append(lambda: _load_misc())

            # per-b double-buffered activations (allocated inside the b loop)
            state = {}
            alt = {"i": 0}  # DVE/Pool alternation for mask adds + oproj evicts

            def proj_mc(b, mc):
                """projections for m-chunk mc of batch b"""
                kaug, qaug, vt = state["kaug"], state["qaug"], state["vt"]
                mco = mc * MC
                qp = [ps.tile([128, MC], f32, tag="qk", bufs=4,
                              name=f"qp{hp}") for hp in range(NHP)]
                kvp = ps.tile([128, MC], f32, tag="ps4", bufs=4, name="kvp")
                KQ = 4  # k-tiles per xt DMA
                DR = mybir.MatmulPerfMode.DoubleRow
                for ktq in range(NKT // KQ):
                    xt = sbp.tile([128, KQ, 2, MC], fp8, tag="xt", bufs=6)
                    nc.sync.dma_start(
                        xt[:], xT_d.ap()[ktq * KQ:(ktq + 1) * KQ, :, :,
                                         b, mco:mco + MC]
                        .rearrange("kt two p m -> p kt two m"))
                    while deferred:
                        deferred.pop(0)()
                    st = (ktq == 0)
                    sp = (ktq == NKT // KQ - 1)
                    groups = [(qp[0], 0), (qp[1], 128), (kvp, DQ)]
                    for dst, g0 in groups:
                        csl = slice(g0, g0 + 128) if g0 < DQ                             else slice(DQ, DQ + DKV)
                        # hi*hi over kt pairs
                        for kp in range(KQ // 2):
                            nc.tensor.matmul(
                                dst[:],
                                wqkv_sb[:, ktq * KQ + 2 * kp:
                                        ktq * KQ + 2 * kp + 2, 1, csl],
                                xt[:, 2 * kp:2 * kp + 2, 0, :],
                                start=st and kp == 0, stop=False,
                                perf_mode=DR)
                        # cross terms (w_lo x_hi + w_hi x_lo) per kt
                        for kq in range(KQ):
                            nc.tensor.matmul(
                                dst[:],
                                wqkv_sb[:, ktq * KQ + kq, :, csl],
                                xt[:, kq, :, :],
                                start=False, stop=sp and kq == KQ - 1,
                                perf_mode=DR)
                # evictions, spread across DVE/ACT so qp frees fast
                # (GPSIMD cannot access PSUM)
                WS = 1.0 / 1024.0  # undo the fp8 weight scaling
                for hp in range(NHP):
                    # even head of the pair: psum rows 0:64 -> qaug rows 0:64
                    nc.vector.tensor_scalar_mul(
                        qaug[2 * hp][0:64, mco:mco + MC], qp[hp][0:64, :], WS)
                    # odd head: rows 64:128, engine-copy then DMA shift
                    qtmp = sbp.tile([128, MC], f32r, tag="qtmp", bufs=4,
                                    name="qtmp")
                    nc.vector.tensor_scalar_mul(qtmp[64:128, :],
                                                qp[hp][64:128, :], WS)
                    nc.sync.dma_start(qaug[2 * hp + 1][0:64, mco:mco + MC],
                                      qtmp[64:128, :])
                nc.vector.tensor_scalar_mul(kaug[0:64, mco:mco + MC],
                                            kvp[0:64, :], WS)
                vtmp = sbp.tile([128, MC], bf16, tag="vtmp", bufs=2,
                                name="vtmp")
                nc.scalar.activation(vtmp[64:128, :], kvp[64:128, :],
                                     mybir.ActivationFunctionType.Copy,
                                     scale=WS)
                nc.sync.dma_start(vt[0:64, mco:mco + MC], vtmp[64:128, :])

            def vtrans(b):
                """transpose vT -> v (vaug), groups of 8 n-tiles per psum"""
                vt, vaug = state["vt"], state["vaug"]
                for g in range((NNT + 7) // 8):
                    nts = range(g * 8, min((g + 1) * 8, NNT))
                    vtp = ps.tile([128, 512], bf16, tag="ps4", bufs=4,
                                  name="vtp")
                    for j, nt in enumerate(nts):
                        nc.tensor.transpose(
                            vtp[:, j * 64:(j + 1) * 64],
                            vt[0:64, nt * 128:(nt + 1) * 128], ident_sb[:])
                    nc.vector.tensor_copy(vaug[:, nts.start:nts.stop, 0:HD],
                                          vtp[:, 0:64 * len(nts)].rearrange(
                                              "p (t d) -> p t d", d=64))

            def attn_head(b, pair, h):
                kaug, qaug, vaug = state["kaug"], state["qaug"], state["vaug"]
                OT = state["OT"]
                hp, odd = h // 2, h % 2
                av = [ps.tile([128, MC], f32, tag="ps4", bufs=4,
                              name=f"av{c}") for c in range(2)]
                nlive = [nt for nt in range(NNT)
                         if live(nt, 2 * pair) or live(nt, 2 * pair + 1)]
                last_nt = nlive[-1]

                def emit_av(nt, c, pt):
                    st = (nt == 0)
                    mc = 2 * pair + c
                    if causal:
                        # columns whose diagonal (last) tile is nt
                        sl = max(0, nt * 128 - mc * MC)
                        sh = min(MC, nt * 128 + 128 - mc * MC)
                        if sh > sl:
                            nc.tensor.matmul(
                                av[c][0:HD + 1, sl:sh],
                                vaug[:, nt, :], pt[:, sl:sh],
                                start=st, stop=True,
                                skip_group_check=True)
                            if sh < MC:
                                nc.tensor.matmul(
                                    av[c][0:HD + 1, sh:MC],
                                    vaug[:, nt, :], pt[:, sh:MC],
                                    start=st, stop=False,
                                    skip_group_check=True)
                        else:
                            nc.tensor.matmul(
                                av[c][0:HD + 1, :], vaug[:, nt, :], pt[:],
                                start=st, stop=False,
                                skip_group_check=True)
                    else:
                        nc.tensor.matmul(
                            av[c][0:HD + 1, :], vaug[:, nt, :], pt[:],
                            start=st, stop=(nt == last_nt))

                # software pipeline: AV runs LAG chunk-steps behind
                # logits/exp so the exp round-trip latency never blocks the
                # in-order PE queue
                LAG = 6
                fifo = []
                for nt in nlive:
                    for c in range(2):
                        mc = 2 * pair + c
                        if not live(nt, mc):
                            continue
                        o = max(0, nt * 128 - mc * MC) if causal else 0
                        qk = ps.tile([128, MC], f32, tag="qk", bufs=4,
                                     name="qk")
                        pt = sbp.tile([128, MC], bf16, tag="pt", bufs=8,
                                      name="pt")
                        cross = crossing(nt, mc)
                        nc.tensor.matmul(
                            qk[:, o:MC],
                            kaug[:, nt * 128:(nt + 1) * 128],
                            qaug[h][:, mc * MC + o:(mc + 1) * MC],
                            start=True, stop=not cross,
                            skip_group_check=cross)
                        if generic_mask:
                            mtile = sbp.tile([128, MC], f32, tag="mt",
                                             name="mt")
                            nc.sync.dma_start(
                                mtile[:],
                                maskT_d.ap()[nt * 128:(nt + 1) * 128,
                                             mc * MC:(mc + 1) * MC])
                            nc.vector.tensor_add(qk[:], qk[:], mtile[:])
                        elif cross:
                            # accumulate the causal pattern on the PE
                            nc.tensor.matmul(
                                qk[:, o:o + 128], ident128_sb[:],
                                mpat_sb[:], start=False, stop=True,
                                skip_group_check=True)
                        nc.scalar.activation(
                            pt[:, o:MC], qk[:, o:MC],
                            mybir.ActivationFunctionType.Exp)
                        fifo.append((nt, c, pt))
                        if len(fifo) > LAG:
                            emit_av(*fifo.pop(0))
                for item in fifo:
                    emit_av(*item)
                # normalize per chunk (chunk 0's chain overlaps chunk 1's
                # remaining AV matmuls): denom row -> broadcast -> divide
                # folded into the bf16 psum eviction
                qdiv = None
                if odd:
                    qdiv = sbp.tile([64, MPAIR], bf16, tag="qdiv", bufs=2,
                                    name="qdiv")
                for c in range(2):
                    cs = slice(c * MC, (c + 1) * MC)
                    srow = sbp.tile([1, MC], f32, tag="srow", bufs=6,
                                    name="srow")
                    nc.vector.tensor_copy(srow[0:1, :], av[c][HD:HD + 1, :])
                    nc.vector.reciprocal(srow[0:1, :], srow[0:1, :])
                    rbc = sbp.tile([128, MC], f32, tag="rbc", bufs=6,
                                   name="rbc")
                    nc.gpsimd.partition_broadcast(rbc[:], srow[0:1, :])
                    dst = qdiv[0:64, cs] if odd else OT[0:64, hp, cs]
                    nc.vector.tensor_mul(dst, av[c][0:64, :], rbc[0:64, :])
                if odd:
                    nc.sync.dma_start(OT[64:128, hp, :], qdiv[0:64, :])

            def oproj_mt(b, pair, OT, mtl, on_act):
                """out-projection for one 128-query tile of a pair; evicts on
                ACT during proj-phase drains (DVE is the proj bottleneck)"""
                mt = pair * (MPAIR // 128) + mtl
                ob = sbp.tile([128, D], bf16, tag="ob", bufs=2, name="ob")
                for ec in range(NEC):
                    op = ps.tile([128, MC], f32, tag="ps4", bufs=4, name="op")
                    for hp in range(NHP):
                        nc.tensor.matmul(
                            op[:],
                            OT[:, hp, mtl * 128:(mtl + 1) * 128],
                            wo_sb[:, hp, ec * MC:(ec + 1) * MC],
                            start=(hp == 0), stop=(hp == NHP - 1))
                    if on_act:
                        nc.scalar.activation(
                            ob[:, ec * MC:(ec + 1) * MC], op[:],
                            mybir.ActivationFunctionType.Copy)
                    else:
                        nc.vector.tensor_copy(ob[:, ec * MC:(ec + 1) * MC],
                                              op[:])
                nc.sync.dma_start(
                    out_d.ap()[b, mt * 128:(mt + 1) * 128, :], ob[:])

            # ---- schedule: proj/attention with interleaved out-proj ------
            pending = []

            def drain(k, on_act=False):
                for _ in range(min(k, len(pending))):
                    pending.pop(0)(on_act)

            for _rep in range(cfg.get("reps", 1)):
                for b in range(B):
                    state["kaug"] = dbl.tile([66, S], f32r, tag="kaug",
                                             bufs=2, name="kaug")
                    nc.scalar.dma_start(state["kaug"][64:66, :],
                                        kaug_d.ap()[:].bitcast(f32r))
                    qaug = []
                    for h in range(HLOC):
                        t = dbl.tile([66, S], f32r, tag=f"qaug{h}", bufs=2,
                                     name=f"qaug{h}")
                        nc.scalar.dma_start(t[64:66, :],
                                            qaug_d.ap()[h].bitcast(f32r))
                        qaug.append(t)
                    state["qaug"] = qaug
                    state["vt"] = dbl.tile([64, S], bf16, tag="vt", bufs=2,
                                           name="vt")
                    state["vaug"] = dbl.tile([128, NNT, HD + 1], bf16,
                                             tag="vaug", bufs=2, name="vaug")
                    nc.vector.memset(state["vaug"][:], 1.0)

                    for mc in range(NMC):
                        proj_mc(b, mc)
                        drain(2)
                    vtrans(b)
                    drain(2)
                    for pair in range(NPAIR):
                        OT = dbl.tile([128, NHP, MPAIR], bf16, tag="OT",
                                      bufs=3, name="OT")
                        state["OT"] = OT
                        for i, h in enumerate(range(HLOC)):
                            attn_head(b, pair, h)
                            drain((0, 2, 3, 3)[i])
                        for mtl in range(MPAIR // 128):
                            pending.append(
                                lambda on_act, b=b, pair=pair, OT=OT,
                                mtl=mtl: oproj_mt(b, pair, OT, mtl, on_act))
                drain(len(pending))

    nc.compile()
    return nc


# ---------------------------------------------------------------------------
# host side
# ---------------------------------------------------------------------------

def _analyze_mask(mask2d, S):
    """classify mask; return (causal, zeros, n_lo, n_hi)"""
    masked = mask2d < -1e8
    if not masked.any():
        return False, True, np.zeros(S, np.int64), np.full(S, S - 1, np.int64)
    tri = np.triu(np.ones((S, S), bool), 1)
    if (masked == tri).all() and (mask2d[~masked] == 0).all():
        return True, False, np.zeros(S, np.int64), np.arange(S)
    allowed = ~masked
    # guard fully-masked rows (keep index 0; softmax row is garbage anyway)
    any_allowed = allowed.any(axis=1)
    idx = np.arange(S)[None, :]
    n_hi = np.where(any_allowed, np.where(allowed, idx, -1).max(axis=1), 0)
    n_lo = np.where(any_allowed, np.where(allowed, idx, S).min(axis=1), 0)
    return False, False, n_lo, n_hi


def _bf16(a):
    import ml_dtypes
    return np.ascontiguousarray(a).astype(ml_dtypes.bfloat16)


def _make_inputs_for_core(core, x, wq, wk, wv, wo, slopes, mask, cfg):
    B, S, D, HLOC, HD = cfg["B"], cfg["S"], cfg["D"], cfg["HLOC"], cfg["HD"]
    h0 = core * HLOC
    kv = core  # one kv head per core
    scale = 1.0 / np.sqrt(HD)

    import ml_dtypes
    FP8 = ml_dtypes.float8_e4m3
    NKT = D // 128
    DQ, DKV = HLOC * HD, 2 * HD
    WPAD = 512

    def _fp8_pair(a):
        hi = a.astype(FP8)
        lo = (a - hi.astype(np.float32)).astype(FP8)
        return hi, lo

    xT = np.ascontiguousarray(x.transpose(2, 0, 1))                 # [D,B,S]
    x_hi, x_lo = _fp8_pair(xT.reshape(NKT, 128, B, S))
    xT2 = np.stack([x_hi, x_lo], axis=1)                   # [kt,2,p,B,S]
    wqkvT = np.concatenate(
        [wq[h0 * HD:(h0 + HLOC) * HD] * scale,
         wk[kv * HD:(kv + 1) * HD],
         wv[kv * HD:(kv + 1) * HD]], axis=0).T                      # [D,384]
    wpad = np.zeros((D, WPAD), np.float32)
    wpad[:, :DQ + DKV] = wqkvT * 1024.0
    w_hi, w_lo = _fp8_pair(wpad.reshape(NKT, 128, WPAD))
    wqkv2 = np.stack([w_lo, w_hi], axis=1)                 # [kt,2,p,512]
    woT = np.ascontiguousarray(wo[:, h0 * HD:(h0 + HLOC) * HD].T)   # [DQ,D]

    n = np.arange(S, dtype=np.float32)
    kaug_ext = np.stack([n, np.ones(S, np.float32)])                # [2,S]

    qaug_ext = np.zeros((HLOC, 2, S), np.float32)
    for i in range(HLOC):
        sl = float(slopes[h0 + i])
        # stabilizer c[m] = max over allowed n of slope*(n-m), clipped >= 0
        c = np.maximum(0.0, np.maximum(sl * (cfg["n_hi"] - n),
                                       sl * (cfg["n_lo"] - n)))
        qaug_ext[i, 0, :] = sl
        qaug_ext[i, 1, :] = -sl * n - c

    ident = np.eye(64, dtype=np.float32)

    ins = {"xT2": xT2, "wqkv2": wqkv2, "woT": _bf16(woT),
           "kaug_ext": kaug_ext, "qaug_ext": qaug_ext,
           "ident": _bf16(ident)}
    if cfg["causal"]:
        ii = np.arange(128)[:, None]
        jj = np.arange(128)[None, :]
        ins["maskpat"] = _bf16(np.where(ii > jj, NEG, 0.0))
        ins["ident128"] = _bf16(np.eye(128))
    if cfg["generic_mask"]:
        ins["maskT"] = np.ascontiguousarray(mask[0, 0].T)
    return ins


def kernel(x, wq, wk, wv, wo, slopes, mask):
    from concourse.bass_utils import run_bass_kernel_spmd

    x = np.asarray(x, dtype=np.float32)
    wq = np.asarray(wq, dtype=np.float32)
    wk = np.asarray(wk, dtype=np.float32)
    wv = np.asarray(wv, dtype=np.float32)
    wo = np.asarray(wo, dtype=np.float32)
    slopes = np.asarray(slopes, dtype=np.float32)
    mask = np.asarray(mask, dtype=np.float32)

    B, S, D = x.shape
    HQ = 32
    HD = D // HQ
    n_cores = 8
    HLOC = HQ // n_cores

    causal, zeros, n_lo, n_hi = _analyze_mask(mask[0, 0], S)
    cfg = dict(B=B, S=S, D=D, HLOC=HLOC, HD=HD, MC=512,
               causal=causal, generic_mask=not (causal or zeros),
               n_lo=n_lo, n_hi=n_hi)

    nc = build_program(cfg)
    in_maps = [_make_inputs_for_core(c, x, wq, wk, wv, wo, slopes, mask, cfg)
               for c in range(n_cores)]
    res = run_bass_kernel_spmd(nc, in_maps, core_ids=list(range(n_cores)))
    out = np.zeros((B, S, D), np.float32)
    for c in range(n_cores):
        out += res.results[c]["out"].astype(np.float32)
    return out


if __name__ == "__main__":
    pass
